# revision 14
# baseline (speedup 1.0000x reference)
"""Trainium2 Bass kernel for the DPRNN block (channel-norm -> unfold ->
4x bidirectional SRU -> conv-transpose -> residual).

Sharding: data-parallel over the B*T=512 sequences; 64 sequences per core.
All weights replicated. Each core runs the full pipeline on its shard.

Layout (per core): sequences live in 128-column blocks (121 valid SRU steps
+ 7 pad columns). Pads carry f=0, b=0 through the scan so a single
tensor_tensor_scan over the whole free dim handles all sequences.
"""
import os
import numpy as np
import ml_dtypes

import concourse.bass as bass
import concourse.mybir as mybir
import concourse.tile as tile
from concourse import bacc
from concourse import bass_utils

F32 = mybir.dt.float32
BF16 = mybir.dt.bfloat16

B, C, T, F_ = 4, 64, 128, 128
H, K = 128, 8
L = F_ - K + 1            # 121
EPS = 1e-8
NCORES = 8
NLOC = (B * T) // NCORES  # 64 sequences per core
NF = NLOC * 128           # 8192
XCOLS = NF + 8            # xn2 / h tiles carry 8 extra cols for shifted reads

DT_H = BF16       # h / xn2 / gate dtype (matmul inputs)
SPAN = 1024       # psum evacuation span (8 seqs)
NSPAN = NF // SPAN

_CACHE = {}


def _build():
    nc = bacc.Bacc("TRN2", target_bir_lowering=False, debug=False)
    AF = mybir.ActivationFunctionType
    OP = mybir.AluOpType

    # ---------------- DRAM tensors ----------------
    u_d = nc.dram_tensor("u", [C, NLOC, F_], F32, kind="ExternalInput").ap()
    w0_d = nc.dram_tensor("w0p", [2, 4, 128, 512], BF16, kind="ExternalInput").ap()
    wi_d = nc.dram_tensor("wip", [3, 2, 2, 128, 512], BF16, kind="ExternalInput").ap()
    cw_d = nc.dram_tensor("cwp", [2, 8, 128, 64], BF16, kind="ExternalInput").ap()
    bf_d = nc.dram_tensor("bfp", [4, 2, 128], F32, kind="ExternalInput").ap()
    br_d = nc.dram_tensor("brp", [4, 2, 128], F32, kind="ExternalInput").ap()
    gm_d = nc.dram_tensor("gm", [C], F32, kind="ExternalInput").ap()
    bt_d = nc.dram_tensor("bt", [C], F32, kind="ExternalInput").ap()
    cb_d = nc.dram_tensor("cb", [C], F32, kind="ExternalInput").ap()
    out_d = nc.dram_tensor("o", [C, NF], F32, kind="ExternalOutput").ap()
    scA_d = nc.dram_tensor("scA", [NLOC, 128], F32).ap()
    scB_d = nc.dram_tensor("scB", [NLOC, 128], F32).ap()

    with tile.TileContext(nc) as tc:
        with tc.tile_pool(name="const", bufs=1) as cp:
            # ---- weights / biases resident in SBUF ----
            w0_t = cp.tile([128, 2 * 4 * 512], BF16)
            w0_v = w0_t[:].rearrange("p (d kp m) -> p d kp m", d=2, kp=4)
            nc.sync.dma_start(w0_v, w0_d.rearrange("d kp p m -> p d kp m"))
            wi_t = cp.tile([128, 3 * 2 * 2 * 512], BF16)
            wi_v = wi_t[:].rearrange("p (i d ct m) -> p i d ct m", i=3, d=2, ct=2)
            nc.sync.dma_start(wi_v, wi_d.rearrange("i d ct p m -> p i d ct m"))
            cw_t = cp.tile([128, 2 * 8 * 64], BF16)
            cw_v = cw_t[:].rearrange("p (ct k m) -> p ct k m", ct=2, k=8)
            nc.sync.dma_start(cw_v, cw_d.rearrange("ct k p m -> p ct k m"))
            bfp_t = cp.tile([128, 8], F32)
            nc.sync.dma_start(bfp_t[:].rearrange("p (i d) -> p i d", i=4), bf_d.rearrange("i d p -> p i d"))
            brp_t = cp.tile([128, 8], F32)
            nc.sync.dma_start(brp_t[:].rearrange("p (i d) -> p i d", i=4), br_d.rearrange("i d p -> p i d"))
            gm1_t = cp.tile([1, C], F32)
            nc.sync.dma_start(gm1_t[:], gm_d.rearrange("(a c) -> a c", a=1))
            bt_t = cp.tile([C, 1], F32)
            nc.sync.dma_start(bt_t[:], bt_d.rearrange("(c a) -> c a", a=1))
            cb_t = cp.tile([C, 1], F32)
            nc.sync.dma_start(cb_t[:], cb_d.rearrange("(c a) -> c a", a=1))

            # ---- long-lived activations ----
            xn_t = cp.tile([C, NF], F32)          # normed input, fp32 (residual)
            xn2_t = cp.tile([128, XCOLS], DT_H)   # [xn ; xn shifted by 1] bf16
            h_t = [cp.tile([128, XCOLS], DT_H, name=f"h{i}") for i in range(4)]  # ping-pong pairs

            nc.gpsimd.memset(xn2_t[:, NF:XCOLS], 0.0)
            nc.gpsimd.memset(xn2_t[64:128, NF - 1:NF], 0.0)
            # zero h tiles once: pads stay zero through all layers (the
            # highway writes only valid columns)
            for i in range(4):
                nc.gpsimd.memset(h_t[i][:], 0.0)

            # ================= channel norm =================
            scA_f = scA_d.rearrange("n f -> (n f)")
            scB_f = scB_d.rearrange("n f -> (n f)")
            with tc.tile_pool(name="normu", bufs=1) as np_:
                u_cn = np_.tile([C, NF], F32)
                nc.sync.dma_start(u_cn[:], u_d.rearrange("c n f -> c (n f)"))
                with (
                    tc.tile_pool(name="normn", bufs=1) as nnp,
                    tc.tile_pool(name="norms", bufs=1) as nsp,
                ):
                    u_nn = nnp.tile([NLOC, C * 128], F32)
                    nc.sync.dma_start(
                        u_nn[:].rearrange("n (c f) -> n c f", f=128),
                        u_d.rearrange("c n f -> n c f"),
                    )
                    # stats over c (innermost of (n, f, c) view)
                    mu_t = nsp.tile([NLOC, 128], F32)
                    s2_t = nsp.tile([NLOC, 128], F32)
                    tmp_t = nsp.tile([NLOC, 128], F32)
                    A_t = nsp.tile([NLOC, 128], F32)
                    B_t = nsp.tile([NLOC, 128], F32)
                    un_v = u_nn[:].rearrange("n (c f) -> n f c", f=128)
                    nc.vector.tensor_reduce(mu_t[:], un_v, axis=mybir.AxisListType.X, op=OP.add)
                    zb_t = nsp.tile([NLOC, 1], F32)
                    nc.vector.memset(zb_t[:], 0.0)
                    nc.scalar.activation(u_nn[:], u_nn[:], AF.Square, bias=zb_t[:, 0:1])
                    nc.vector.tensor_reduce(s2_t[:], un_v, axis=mybir.AxisListType.X, op=OP.add)
                    nc.vector.tensor_scalar_mul(mu_t[:], mu_t[:], 1.0 / C)
                    nc.vector.tensor_scalar_mul(s2_t[:], s2_t[:], 1.0 / C)
                    nc.vector.tensor_mul(tmp_t[:], mu_t[:], mu_t[:])
                    nc.vector.tensor_sub(s2_t[:], s2_t[:], tmp_t[:])  # var
                    eps_t = nsp.tile([NLOC, 1], F32)
                    nc.vector.memset(eps_t[:], EPS)
                    nc.scalar.activation(tmp_t[:], s2_t[:], AF.Sqrt, bias=eps_t[:, 0:1])
                    nc.vector.reciprocal(A_t[:], tmp_t[:])            # rstd
                    nc.vector.scalar_tensor_tensor(
                        B_t[:], mu_t[:], -1.0, A_t[:], op0=OP.mult, op1=OP.mult
                    )
                    # stats to DRAM, re-read as flat rows per chunk
                    nc.sync.dma_start(scA_d, A_t[:])
                    nc.sync.dma_start(scB_d, B_t[:])

                # broadcast along c with gamma folded in, then apply
                CH = 1024
                with (
                    tc.tile_pool(name="normab", bufs=4) as nab,
                    tc.tile_pool(name="normps", bufs=2, space="PSUM") as npp,
                ):
                    for ch in range(NF // CH):
                        ag = npp.tile([C, CH], F32, tag="ag")
                        bg = npp.tile([C, CH], F32, tag="bg")
                        for h2 in range(CH // 512):
                            lo = ch * CH + h2 * 512
                            a1 = nab.tile([1, 512], F32, tag="a1")
                            b1 = nab.tile([1, 512], F32, tag="b1")
                            nc.sync.dma_start(
                                a1[:], scA_f[lo:lo + 512].rearrange("(a x) -> a x", a=1))
                            nc.sync.dma_start(
                                b1[:], scB_f[lo:lo + 512].rearrange("(a x) -> a x", a=1))
                            nc.tensor.matmul(ag[:, h2 * 512:(h2 + 1) * 512], gm1_t[:],
                                             a1[:], start=True, stop=True)
                            nc.tensor.matmul(bg[:, h2 * 512:(h2 + 1) * 512], gm1_t[:],
                                             b1[:], start=True, stop=True)
                        sl = slice(ch * CH, (ch + 1) * CH)
                        nc.vector.tensor_mul(xn_t[:, sl], u_cn[:, sl], ag[:])
                        nc.vector.scalar_tensor_tensor(
                            xn_t[:, sl], xn_t[:, sl], bt_t[:, 0:1], bg[:],
                            op0=OP.add, op1=OP.add,
                        )
                        # bf16 copies into xn2 (rows 0:64 plain, 64:128 shifted by 1)
                        nc.scalar.copy(xn2_t[0:64, sl], xn_t[:, sl])
                        nc.scalar.copy(
                            xn2_t[64:128, ch * CH:(ch + 1) * CH - 1],
                            xn_t[:, ch * CH + 1:(ch + 1) * CH],
                        )
                    # chunk-boundary columns of the shifted copy (read the
                    # first col of the next chunk, so emitted after the loop)
                    xn_bv = xn_t[:].rearrange("p (a b) -> p a b", b=CH)
                    x2_bv = xn2_t[64:128, 0:NF].rearrange("p (a b) -> p a b", b=CH)
                    nc.scalar.copy(
                        x2_bv[:, 0:NF // CH - 1, CH - 1:CH],
                        xn_bv[:, 1:NF // CH, 0:1],
                    )

            # ================= SRU layers =================
            sig = AF.Sigmoid
            with (
                tc.tile_pool(name="gates", bufs=1) as gp,
                tc.tile_pool(name="lps", bufs=1, space="PSUM") as pp,
            ):
                for li in range(4):
                    if li == 0:
                        hin = None
                        nct = 4
                    else:
                        hin = [h_t[2 * ((li - 1) % 2)], h_t[2 * ((li - 1) % 2) + 1]]
                        nct = 2
                    hout = [h_t[2 * (li % 2)], h_t[2 * (li % 2) + 1]]
                    ooff = 8 if li == 3 else 0
                    for half in range(2):
                        # per-direction gate tiles (g/f, b, r, hp)
                        gt = []
                        for d in range(2):
                            f_t = gp.tile([128, NF // 2], DT_H, name=f"f{d}", tag=f"f{d}")
                            b_t = gp.tile([128, NF // 2], DT_H, name=f"b{d}", tag=f"b{d}")
                            r_t = gp.tile([128, NF // 2], DT_H, name=f"r{d}", tag=f"r{d}")
                            w_t = gp.tile([128, NF // 2], DT_H, name=f"w{d}", tag=f"w{d}")
                            gt.append((f_t, b_t, r_t, w_t))
                        # interleave the two directions span-by-span so PE
                        # stays dense while the other direction evacuates
                        for s4 in range(NSPAN // 2):
                            span = half * (NSPAN // 2) + s4
                            for d in range(2):
                                f_t, b_t, r_t, w_t = gt[d]
                                bcol = bfp_t[:, 2 * li + d:2 * li + d + 1]
                                rcol = brp_t[:, 2 * li + d:2 * li + d + 1]
                                pst = [pp.tile([128, SPAN], F32, name=t, tag=t)
                                       for t in ("z", "f", "r", "hp")]
                                for o in range(4):
                                    for h2 in range(SPAN // 512):
                                        osl = pst[o][:, h2 * 512:(h2 + 1) * 512]
                                        base = span * SPAN + h2 * 512
                                        for ct in range(nct):
                                            if li == 0:
                                                rhs = xn2_t[:, base + 2 * ct:base + 2 * ct + 512]
                                                lhsT = w0_v[:, d, ct, o * 128:(o + 1) * 128]
                                            else:
                                                rhs = hin[ct][:, base:base + 512]
                                                lhsT = wi_v[:, li - 1, d, ct, o * 128:(o + 1) * 128]
                                            nc.tensor.matmul(
                                                osl, lhsT, rhs,
                                                start=(ct == 0), stop=(ct == nct - 1),
                                            )
                                # evacuate span, full 128-blocks. d=1 stores each
                                # block reversed (pads land at l' in [0,7)).
                                ssl = slice(s4 * SPAN, (s4 + 1) * SPAN)
                                if d == 0:
                                    srcs = [pq[:] for pq in pst]
                                else:
                                    srcs = [pq[:].rearrange("p (n l) -> p n l", l=128)[:, :, ::-1]
                                            for pq in pst]
                                # g = 1-f = sigmoid(-(x+bf)); bfp holds -bf
                                nc.scalar.activation(f_t[:, ssl], srcs[1], sig,
                                                     bias=bcol, scale=-1.0)
                                nc.scalar.activation(r_t[:, ssl], srcs[2], sig, bias=rcol)
                                nc.scalar.copy(b_t[:, ssl], srcs[0])   # z
                                nc.scalar.copy(w_t[:, ssl], srcs[3])   # hp
                                # b'' = g*z (in place over z; reads g before the
                                # 1-g pass below overwrites it)
                                nc.vector.tensor_mul(b_t[:, ssl], f_t[:, ssl], b_t[:, ssl])
                        for d in range(2):
                            f_t, b_t, r_t, w_t = gt[d]
                            f_v = f_t[:].rearrange("p (n l) -> p n l", l=128)
                            b_v = b_t[:].rearrange("p (n l) -> p n l", l=128)
                            # f = 1 - g  (tensor_scalar, 4x mode)
                            nc.vector.tensor_scalar(f_t[:], f_t[:], -1.0, 1.0,
                                                    op0=OP.mult, op1=OP.add)
                            # pads reset the scan carry between sequences
                            pads = slice(121, 128) if d == 0 else slice(0, 7)
                            nc.gpsimd.memset(f_v[:, :, pads], 0.0)
                            nc.gpsimd.memset(b_v[:, :, pads], 0.0)
                            # c = f*c + (1-f)*z
                            nc.vector.tensor_tensor_scan(
                                b_t[:], f_t[:], b_t[:], 0.0,
                                op0=OP.mult, op1=OP.add,
                            )
                            # highway: out = r*(cs-hp) + hp
                            nc.vector.tensor_sub(f_t[:], b_t[:], w_t[:])
                            nc.vector.tensor_mul(r_t[:], r_t[:], f_t[:])
                            hov = hout[d][:, ooff:ooff + NF].rearrange(
                                "p (n l) -> p n l", l=128
                            )
                            dst = hov[:, half * 32:half * 32 + 32, :]
                            if d == 1:
                                dst = dst[:, :, ::-1]
                            r_v = r_t[:].rearrange("p (n l) -> p n l", l=128)
                            w_v = w_t[:].rearrange("p (n l) -> p n l", l=128)
                            nc.vector.tensor_add(dst, r_v[:, :, :], w_v[:, :, :])

            # ================= transposed conv + residual =================
            h4 = [h_t[2], h_t[3]]  # layer 3 writes pair B at offset 8
            for t4 in h4:
                v = t4[:, 0:NF].rearrange("p (n l) -> p n l", l=128)
                nc.gpsimd.memset(t4[:, 0:8], 0.0)
                nc.gpsimd.memset(v[:, 1:64, 1:8], 0.0)
                nc.gpsimd.memset(t4[:, NF + 1:XCOLS], 0.0)
            with (
                tc.tile_pool(name="cvp", bufs=4, space="PSUM") as cvp,
                tc.tile_pool(name="osp", bufs=2) as osp,
            ):
                for span in range(NSPAN):
                    c_ps = cvp.tile([C, SPAN], F32, tag="c")
                    for h2 in range(SPAN // 512):
                        osl = c_ps[:, h2 * 512:(h2 + 1) * 512]
                        base = span * SPAN + h2 * 512
                        mm = 0
                        for ct in range(2):
                            for k in range(8):
                                rhs = h4[ct][:, 8 - k + base:8 - k + base + 512]
                                nc.tensor.matmul(
                                    osl, cw_v[:, ct, k, :], rhs,
                                    start=(mm == 0), stop=(mm == 15),
                                )
                                mm += 1
                    o_t = osp.tile([C, SPAN], F32, tag="o")
                    sl = slice(span * SPAN, (span + 1) * SPAN)
                    nc.vector.scalar_tensor_tensor(
                        o_t[:], c_ps[:], cb_t[:, 0:1], xn_t[:, sl],
                        op0=OP.add, op1=OP.add,
                    )
                    nc.sync.dma_start(out_d[:, sl], o_t[:])

    nc.compile()
    return nc


def _prep_weights(W0, Ws, convW):
    w0r = W0.reshape(C, K, 2, 4 * H)
    w0p = np.zeros((2, 4, 128, 512), np.float32)
    for d in range(2):
        for kp in range(4):
            w0p[d, kp, 0:64] = w0r[:, 2 * kp, d]
            w0p[d, kp, 64:128] = w0r[:, 2 * kp + 1, d]
    wip = np.zeros((3, 2, 2, 128, 512), np.float32)
    for i in range(3):
        for d in range(2):
            for ct in range(2):
                wip[i, d, ct] = Ws[i][ct * 128:(ct + 1) * 128, d]
    cwp = np.zeros((2, 8, 128, C), np.float32)
    for ct in range(2):
        for k in range(8):
            cwp[ct, k] = convW[ct * 128:(ct + 1) * 128, :, k]
    bf16 = ml_dtypes.bfloat16
    return w0p.astype(bf16), wip.astype(bf16), cwp.astype(bf16)


def kernel(**inputs):
    inputs = {k: np.asarray(v) for k, v in inputs.items()}
    x = inputs["x"].astype(np.float32)
    xs = np.ascontiguousarray(
        x.transpose(0, 2, 1, 3).reshape(B * T, C, F_)
    )  # (512, C, F)

    w0p, wip, cwp = _prep_weights(
        inputs["W0"].astype(np.float32),
        [inputs[f"W{i}"].astype(np.float32) for i in (1, 2, 3)],
        inputs["convW"].astype(np.float32),
    )
    bfp = -np.stack([inputs[f"bf{i}"] for i in range(4)]).astype(np.float32)
    brp = np.stack([inputs[f"br{i}"] for i in range(4)]).astype(np.float32)
    gm = inputs["gamma"].reshape(C).astype(np.float32)
    bt = inputs["beta"].reshape(C).astype(np.float32)
    cb = inputs["convb"].reshape(C).astype(np.float32)

    if "nc" not in _CACHE:
        _CACHE["nc"] = _build()
    nc = _CACHE["nc"]

    shared = {"w0p": w0p, "wip": wip, "cwp": cwp, "bfp": bfp, "brp": brp,
              "gm": gm, "bt": bt, "cb": cb}
    in_maps = []
    for core in range(NCORES):
        u = np.ascontiguousarray(
            xs[core * NLOC:(core + 1) * NLOC].transpose(1, 0, 2)
        )  # (C, NLOC, F)
        in_maps.append({"u": u, **shared})

    trace = bool(os.environ.get("KBENCH_TRACE"))
    res = bass_utils.run_bass_kernel_spmd(
        nc, in_maps, list(range(NCORES)), trace=trace,
        tmpdir=os.environ.get("KBENCH_TMPDIR"),
    )
    _CACHE["last_result"] = res

    full = np.concatenate(
        [res.results[i]["o"].reshape(C, NLOC, F_) for i in range(NCORES)], axis=1
    )  # (C, 512, F)
    out = full.transpose(1, 0, 2).reshape(B, T, C, F_).transpose(0, 2, 1, 3)
    return np.ascontiguousarray(out.astype(np.float32))


# revision 15
# speedup vs baseline: 1.2108x; 1.2108x over previous
"""Trainium2 Bass kernel for the DPRNN block (channel-norm -> unfold ->
4x bidirectional SRU -> conv-transpose -> residual).

Sharding: data-parallel over the B*T=512 sequences; 64 sequences per core.
All weights replicated. Each core runs the full pipeline on its shard.

Layout (per core): sequences live in 128-column blocks (121 valid SRU steps
+ 7 pad columns). Pads carry f=0, b=0 through the scan so a single
tensor_tensor_scan over the whole free dim handles all sequences.
"""
import os
import numpy as np
import ml_dtypes

import concourse.bass as bass
import concourse.mybir as mybir
import concourse.tile as tile
from concourse import bacc
from concourse import bass_utils

F32 = mybir.dt.float32
BF16 = mybir.dt.bfloat16

B, C, T, F_ = 4, 64, 128, 128
H, K = 128, 8
L = F_ - K + 1            # 121
EPS = 1e-8
NCORES = 8
NLOC = (B * T) // NCORES  # 64 sequences per core
NF = NLOC * 128           # 8192
XCOLS = NF + 8            # xn2 / h tiles carry 8 extra cols for shifted reads

DT_H = BF16       # h / xn2 / gate dtype (matmul inputs)
SPAN = 1024       # psum evacuation span (8 seqs)
NSPAN = NF // SPAN

_CACHE = {}


def _build():
    nc = bacc.Bacc("TRN2", target_bir_lowering=False, debug=False)
    AF = mybir.ActivationFunctionType
    OP = mybir.AluOpType

    # ---------------- DRAM tensors ----------------
    u_d = nc.dram_tensor("u", [C, NLOC, F_], F32, kind="ExternalInput").ap()
    w0_d = nc.dram_tensor("w0p", [2, 4, 128, 512], BF16, kind="ExternalInput").ap()
    wi_d = nc.dram_tensor("wip", [3, 2, 2, 128, 512], BF16, kind="ExternalInput").ap()
    cw_d = nc.dram_tensor("cwp", [2, 8, 128, 64], BF16, kind="ExternalInput").ap()
    bf_d = nc.dram_tensor("bfp", [4, 2, 128], F32, kind="ExternalInput").ap()
    br_d = nc.dram_tensor("brp", [4, 2, 128], F32, kind="ExternalInput").ap()
    gm_d = nc.dram_tensor("gm", [C], F32, kind="ExternalInput").ap()
    bt_d = nc.dram_tensor("bt", [C], F32, kind="ExternalInput").ap()
    cb_d = nc.dram_tensor("cb", [C], F32, kind="ExternalInput").ap()
    out_d = nc.dram_tensor("o", [C, NF], F32, kind="ExternalOutput").ap()
    scA_d = nc.dram_tensor("scA", [NLOC, 128], F32).ap()
    scB_d = nc.dram_tensor("scB", [NLOC, 128], F32).ap()

    with tile.TileContext(nc) as tc:
        with tc.tile_pool(name="const", bufs=1) as cp:
            # ---- weights / biases resident in SBUF ----
            w0_t = cp.tile([128, 2 * 4 * 512], BF16)
            w0_v = w0_t[:].rearrange("p (d kp m) -> p d kp m", d=2, kp=4)
            nc.sync.dma_start(w0_v, w0_d.rearrange("d kp p m -> p d kp m"))
            wi_t = cp.tile([128, 3 * 2 * 2 * 512], BF16)
            wi_v = wi_t[:].rearrange("p (i d ct m) -> p i d ct m", i=3, d=2, ct=2)
            nc.sync.dma_start(wi_v, wi_d.rearrange("i d ct p m -> p i d ct m"))
            cw_t = cp.tile([128, 2 * 8 * 64], BF16)
            cw_v = cw_t[:].rearrange("p (ct k m) -> p ct k m", ct=2, k=8)
            nc.sync.dma_start(cw_v, cw_d.rearrange("ct k p m -> p ct k m"))
            bfp_t = cp.tile([128, 8], F32)
            nc.sync.dma_start(bfp_t[:].rearrange("p (i d) -> p i d", i=4), bf_d.rearrange("i d p -> p i d"))
            brp_t = cp.tile([128, 8], F32)
            nc.sync.dma_start(brp_t[:].rearrange("p (i d) -> p i d", i=4), br_d.rearrange("i d p -> p i d"))
            gm1_t = cp.tile([1, C], F32)
            nc.sync.dma_start(gm1_t[:], gm_d.rearrange("(a c) -> a c", a=1))
            bt_t = cp.tile([C, 1], F32)
            nc.sync.dma_start(bt_t[:], bt_d.rearrange("(c a) -> c a", a=1))
            cb_t = cp.tile([C, 1], F32)
            nc.sync.dma_start(cb_t[:], cb_d.rearrange("(c a) -> c a", a=1))

            # ---- long-lived activations ----
            xn_t = cp.tile([C, NF], F32)          # normed input, fp32 (residual)
            xn2_t = cp.tile([128, XCOLS], DT_H)   # [xn ; xn shifted by 1] bf16
            h_t = [cp.tile([128, XCOLS], DT_H, name=f"h{i}") for i in range(4)]  # ping-pong pairs

            nc.gpsimd.memset(xn2_t[:, NF:XCOLS], 0.0)
            nc.gpsimd.memset(xn2_t[64:128, NF - 1:NF], 0.0)
            # zero h tiles once: pads stay zero through all layers (the
            # highway writes only valid columns)
            for i in range(4):
                nc.gpsimd.memset(h_t[i][:], 0.0)

            # ================= channel norm =================
            scA_f = scA_d.rearrange("n f -> (n f)")
            scB_f = scB_d.rearrange("n f -> (n f)")
            with tc.tile_pool(name="normu", bufs=1) as np_:
                u_cn = np_.tile([C, NF], F32)
                nc.sync.dma_start(u_cn[:], u_d.rearrange("c n f -> c (n f)"))
                with (
                    tc.tile_pool(name="normn", bufs=1) as nnp,
                    tc.tile_pool(name="norms", bufs=1) as nsp,
                ):
                    u_nn = nnp.tile([NLOC, C * 128], F32)
                    nc.sync.dma_start(
                        u_nn[:].rearrange("n (c f) -> n c f", f=128),
                        u_d.rearrange("c n f -> n c f"),
                    )
                    # stats over c (innermost of (n, f, c) view)
                    mu_t = nsp.tile([NLOC, 128], F32)
                    s2_t = nsp.tile([NLOC, 128], F32)
                    tmp_t = nsp.tile([NLOC, 128], F32)
                    A_t = nsp.tile([NLOC, 128], F32)
                    B_t = nsp.tile([NLOC, 128], F32)
                    un_v = u_nn[:].rearrange("n (c f) -> n f c", f=128)
                    nc.vector.tensor_reduce(mu_t[:], un_v, axis=mybir.AxisListType.X, op=OP.add)
                    zb_t = nsp.tile([NLOC, 1], F32)
                    nc.vector.memset(zb_t[:], 0.0)
                    nc.scalar.activation(u_nn[:], u_nn[:], AF.Square, bias=zb_t[:, 0:1])
                    nc.vector.tensor_reduce(s2_t[:], un_v, axis=mybir.AxisListType.X, op=OP.add)
                    nc.vector.tensor_scalar_mul(mu_t[:], mu_t[:], 1.0 / C)
                    nc.vector.tensor_scalar_mul(s2_t[:], s2_t[:], 1.0 / C)
                    nc.vector.tensor_mul(tmp_t[:], mu_t[:], mu_t[:])
                    nc.vector.tensor_sub(s2_t[:], s2_t[:], tmp_t[:])  # var
                    eps_t = nsp.tile([NLOC, 1], F32)
                    nc.vector.memset(eps_t[:], EPS)
                    nc.scalar.activation(tmp_t[:], s2_t[:], AF.Sqrt, bias=eps_t[:, 0:1])
                    nc.vector.reciprocal(A_t[:], tmp_t[:])            # rstd
                    nc.vector.scalar_tensor_tensor(
                        B_t[:], mu_t[:], -1.0, A_t[:], op0=OP.mult, op1=OP.mult
                    )
                    # stats to DRAM, re-read as flat rows per chunk
                    nc.sync.dma_start(scA_d, A_t[:])
                    nc.sync.dma_start(scB_d, B_t[:])

                # broadcast along c with gamma folded in, then apply
                CH = 1024
                with (
                    tc.tile_pool(name="normab", bufs=4) as nab,
                    tc.tile_pool(name="normps", bufs=2, space="PSUM") as npp,
                ):
                    for ch in range(NF // CH):
                        ag = npp.tile([C, CH], F32, tag="ag")
                        bg = npp.tile([C, CH], F32, tag="bg")
                        for h2 in range(CH // 512):
                            lo = ch * CH + h2 * 512
                            a1 = nab.tile([1, 512], F32, tag="a1")
                            b1 = nab.tile([1, 512], F32, tag="b1")
                            nc.sync.dma_start(
                                a1[:], scA_f[lo:lo + 512].rearrange("(a x) -> a x", a=1))
                            nc.sync.dma_start(
                                b1[:], scB_f[lo:lo + 512].rearrange("(a x) -> a x", a=1))
                            nc.tensor.matmul(ag[:, h2 * 512:(h2 + 1) * 512], gm1_t[:],
                                             a1[:], start=True, stop=True)
                            nc.tensor.matmul(bg[:, h2 * 512:(h2 + 1) * 512], gm1_t[:],
                                             b1[:], start=True, stop=True)
                        sl = slice(ch * CH, (ch + 1) * CH)
                        nc.vector.tensor_mul(xn_t[:, sl], u_cn[:, sl], ag[:])
                        nc.vector.scalar_tensor_tensor(
                            xn_t[:, sl], xn_t[:, sl], bt_t[:, 0:1], bg[:],
                            op0=OP.add, op1=OP.add,
                        )
                        # bf16 copies into xn2 (rows 0:64 plain, 64:128 shifted by 1)
                        nc.scalar.copy(xn2_t[0:64, sl], xn_t[:, sl])
                        nc.scalar.copy(
                            xn2_t[64:128, ch * CH:(ch + 1) * CH - 1],
                            xn_t[:, ch * CH + 1:(ch + 1) * CH],
                        )
                    # chunk-boundary columns of the shifted copy (read the
                    # first col of the next chunk, so emitted after the loop)
                    xn_bv = xn_t[:].rearrange("p (a b) -> p a b", b=CH)
                    x2_bv = xn2_t[64:128, 0:NF].rearrange("p (a b) -> p a b", b=CH)
                    nc.scalar.copy(
                        x2_bv[:, 0:NF // CH - 1, CH - 1:CH],
                        xn_bv[:, 1:NF // CH, 0:1],
                    )

            # ================= SRU layers =================
            sig = AF.Sigmoid
            with (
                tc.tile_pool(name="gates", bufs=2) as gp,
                tc.tile_pool(name="lps", bufs=1, space="PSUM") as pp,
            ):
                for li in range(4):
                    if li == 0:
                        hin = None
                        nct = 4
                    else:
                        hin = [h_t[2 * ((li - 1) % 2)], h_t[2 * ((li - 1) % 2) + 1]]
                        nct = 2
                    hout = [h_t[2 * (li % 2)], h_t[2 * (li % 2) + 1]]
                    ooff = 8 if li == 3 else 0
                    for d in range(2):
                        bcol = bfp_t[:, 2 * li + d:2 * li + d + 1]
                        rcol = brp_t[:, 2 * li + d:2 * li + d + 1]
                        for half in range(2):
                            f_t = gp.tile([128, NF // 2], DT_H, tag="f")
                            b_t = gp.tile([128, NF // 2], DT_H, tag="b")
                            r_t = gp.tile([128, NF // 2], DT_H, tag="r")
                            w_t = gp.tile([128, NF // 2], DT_H, tag="w")
                            f_v = f_t[:].rearrange("p (n l) -> p n l", l=128)
                            b_v = b_t[:].rearrange("p (n l) -> p n l", l=128)
                            for s4 in range(NSPAN // 2):
                                span = half * (NSPAN // 2) + s4
                                pst = [pp.tile([128, SPAN], F32, name=t, tag=t)
                                       for t in ("z", "f", "r", "hp")]
                                for o in range(4):
                                    for h2 in range(SPAN // 512):
                                        osl = pst[o][:, h2 * 512:(h2 + 1) * 512]
                                        base = span * SPAN + h2 * 512
                                        for ct in range(nct):
                                            if li == 0:
                                                rhs = xn2_t[:, base + 2 * ct:base + 2 * ct + 512]
                                                lhsT = w0_v[:, d, ct, o * 128:(o + 1) * 128]
                                            else:
                                                rhs = hin[ct][:, base:base + 512]
                                                lhsT = wi_v[:, li - 1, d, ct, o * 128:(o + 1) * 128]
                                            nc.tensor.matmul(
                                                osl, lhsT, rhs,
                                                start=(ct == 0), stop=(ct == nct - 1),
                                            )
                                # evacuate span, full 128-blocks. d=1 stores each
                                # block reversed (pads land at l' in [0,7)).
                                ssl = slice(s4 * SPAN, (s4 + 1) * SPAN)
                                if d == 0:
                                    srcs = [pq[:] for pq in pst]
                                else:
                                    srcs = [pq[:].rearrange("p (n l) -> p n l", l=128)[:, :, ::-1]
                                            for pq in pst]
                                # g = 1-f = sigmoid(-(x+bf)); bfp holds -bf
                                nc.scalar.activation(f_t[:, ssl], srcs[1], sig,
                                                     bias=bcol, scale=-1.0)
                                nc.scalar.activation(r_t[:, ssl], srcs[2], sig, bias=rcol)
                                nc.scalar.copy(b_t[:, ssl], srcs[0])   # z
                                nc.scalar.copy(w_t[:, ssl], srcs[3])   # hp
                                # b'' = g*z (in place over z; must read g before
                                # the 1-g pass below overwrites it)
                                nc.vector.tensor_mul(b_t[:, ssl], f_t[:, ssl], b_t[:, ssl])
                            # f = 1 - g  (tensor_scalar, 4x mode)
                            nc.vector.tensor_scalar(f_t[:], f_t[:], -1.0, 1.0,
                                                    op0=OP.mult, op1=OP.add)
                            # pads reset the scan carry between sequences
                            pads = slice(121, 128) if d == 0 else slice(0, 7)
                            nc.gpsimd.memset(f_v[:, :, pads], 0.0)
                            nc.gpsimd.memset(b_v[:, :, pads], 0.0)
                            # c = f*c + (1-f)*z
                            nc.vector.tensor_tensor_scan(
                                b_t[:], f_t[:], b_t[:], 0.0,
                                op0=OP.mult, op1=OP.add,
                            )
                            # highway: out = r*(cs-hp) + hp
                            nc.vector.tensor_sub(f_t[:], b_t[:], w_t[:])
                            nc.vector.tensor_mul(r_t[:], r_t[:], f_t[:])
                            hov = hout[d][:, ooff:ooff + NF].rearrange(
                                "p (n l) -> p n l", l=128
                            )
                            dst = hov[:, half * 32:half * 32 + 32, :]
                            if d == 1:
                                dst = dst[:, :, ::-1]
                            r_v = r_t[:].rearrange("p (n l) -> p n l", l=128)
                            w_v = w_t[:].rearrange("p (n l) -> p n l", l=128)
                            nc.vector.tensor_add(dst, r_v[:, :, :], w_v[:, :, :])

            # ================= transposed conv + residual =================
            h4 = [h_t[2], h_t[3]]  # layer 3 writes pair B at offset 8
            for t4 in h4:
                v = t4[:, 0:NF].rearrange("p (n l) -> p n l", l=128)
                nc.gpsimd.memset(t4[:, 0:8], 0.0)
                nc.gpsimd.memset(v[:, 1:64, 1:8], 0.0)
                nc.gpsimd.memset(t4[:, NF + 1:XCOLS], 0.0)
            with (
                tc.tile_pool(name="cvp", bufs=4, space="PSUM") as cvp,
                tc.tile_pool(name="osp", bufs=2) as osp,
            ):
                for span in range(NSPAN):
                    c_ps = cvp.tile([C, SPAN], F32, tag="c")
                    for h2 in range(SPAN // 512):
                        osl = c_ps[:, h2 * 512:(h2 + 1) * 512]
                        base = span * SPAN + h2 * 512
                        mm = 0
                        for ct in range(2):
                            for k in range(8):
                                rhs = h4[ct][:, 8 - k + base:8 - k + base + 512]
                                nc.tensor.matmul(
                                    osl, cw_v[:, ct, k, :], rhs,
                                    start=(mm == 0), stop=(mm == 15),
                                )
                                mm += 1
                    o_t = osp.tile([C, SPAN], F32, tag="o")
                    sl = slice(span * SPAN, (span + 1) * SPAN)
                    nc.vector.scalar_tensor_tensor(
                        o_t[:], c_ps[:], cb_t[:, 0:1], xn_t[:, sl],
                        op0=OP.add, op1=OP.add,
                    )
                    nc.sync.dma_start(out_d[:, sl], o_t[:])

    nc.compile()
    return nc


def _prep_weights(W0, Ws, convW):
    w0r = W0.reshape(C, K, 2, 4 * H)
    w0p = np.zeros((2, 4, 128, 512), np.float32)
    for d in range(2):
        for kp in range(4):
            w0p[d, kp, 0:64] = w0r[:, 2 * kp, d]
            w0p[d, kp, 64:128] = w0r[:, 2 * kp + 1, d]
    wip = np.zeros((3, 2, 2, 128, 512), np.float32)
    for i in range(3):
        for d in range(2):
            for ct in range(2):
                wip[i, d, ct] = Ws[i][ct * 128:(ct + 1) * 128, d]
    cwp = np.zeros((2, 8, 128, C), np.float32)
    for ct in range(2):
        for k in range(8):
            cwp[ct, k] = convW[ct * 128:(ct + 1) * 128, :, k]
    bf16 = ml_dtypes.bfloat16
    return w0p.astype(bf16), wip.astype(bf16), cwp.astype(bf16)


def kernel(**inputs):
    inputs = {k: np.asarray(v) for k, v in inputs.items()}
    x = inputs["x"].astype(np.float32)
    xs = np.ascontiguousarray(
        x.transpose(0, 2, 1, 3).reshape(B * T, C, F_)
    )  # (512, C, F)

    w0p, wip, cwp = _prep_weights(
        inputs["W0"].astype(np.float32),
        [inputs[f"W{i}"].astype(np.float32) for i in (1, 2, 3)],
        inputs["convW"].astype(np.float32),
    )
    bfp = -np.stack([inputs[f"bf{i}"] for i in range(4)]).astype(np.float32)
    brp = np.stack([inputs[f"br{i}"] for i in range(4)]).astype(np.float32)
    gm = inputs["gamma"].reshape(C).astype(np.float32)
    bt = inputs["beta"].reshape(C).astype(np.float32)
    cb = inputs["convb"].reshape(C).astype(np.float32)

    if "nc" not in _CACHE:
        _CACHE["nc"] = _build()
    nc = _CACHE["nc"]

    shared = {"w0p": w0p, "wip": wip, "cwp": cwp, "bfp": bfp, "brp": brp,
              "gm": gm, "bt": bt, "cb": cb}
    in_maps = []
    for core in range(NCORES):
        u = np.ascontiguousarray(
            xs[core * NLOC:(core + 1) * NLOC].transpose(1, 0, 2)
        )  # (C, NLOC, F)
        in_maps.append({"u": u, **shared})

    trace = bool(os.environ.get("KBENCH_TRACE"))
    res = bass_utils.run_bass_kernel_spmd(
        nc, in_maps, list(range(NCORES)), trace=trace,
        tmpdir=os.environ.get("KBENCH_TMPDIR"),
    )
    _CACHE["last_result"] = res

    full = np.concatenate(
        [res.results[i]["o"].reshape(C, NLOC, F_) for i in range(NCORES)], axis=1
    )  # (C, 512, F)
    out = full.transpose(1, 0, 2).reshape(B, T, C, F_).transpose(0, 2, 1, 3)
    return np.ascontiguousarray(out.astype(np.float32))


# revision 18
# speedup vs baseline: 1.2373x; 1.0219x over previous
"""Trainium2 Bass kernel for the DPRNN block (channel-norm -> unfold ->
4x bidirectional SRU -> conv-transpose -> residual).

Sharding: data-parallel over the B*T=512 sequences; 64 sequences per core.
All weights replicated. Each core runs the full pipeline on its shard.

Layout (per core): sequences live in 128-column blocks (121 valid SRU steps
+ 7 pad columns). Pads carry f=0, b=0 through the scan so a single
tensor_tensor_scan over the whole free dim handles all sequences.
"""
import os
import numpy as np
import ml_dtypes

import concourse.bass as bass
import concourse.mybir as mybir
import concourse.tile as tile
from concourse import bacc
from concourse import bass_utils

F32 = mybir.dt.float32
BF16 = mybir.dt.bfloat16

B, C, T, F_ = 4, 64, 128, 128
H, K = 128, 8
L = F_ - K + 1            # 121
EPS = 1e-8
NCORES = 8
NLOC = (B * T) // NCORES  # 64 sequences per core
NF = NLOC * 128           # 8192
XCOLS = NF + 8            # xn2 / h tiles carry 8 extra cols for shifted reads

DT_H = BF16       # h / xn2 / gate dtype (matmul inputs)
SPAN = 1024       # psum evacuation span (8 seqs)
NSPAN = NF // SPAN

_CACHE = {}


def _build():
    nc = bacc.Bacc("TRN2", target_bir_lowering=False, debug=False)
    AF = mybir.ActivationFunctionType
    OP = mybir.AluOpType

    # ---------------- DRAM tensors ----------------
    u_d = nc.dram_tensor("u", [C, NLOC, F_], F32, kind="ExternalInput").ap()
    w0_d = nc.dram_tensor("w0p", [2, 4, 128, 512], BF16, kind="ExternalInput").ap()
    wi_d = nc.dram_tensor("wip", [3, 2, 2, 128, 512], BF16, kind="ExternalInput").ap()
    cw_d = nc.dram_tensor("cwp", [2, 8, 128, 64], BF16, kind="ExternalInput").ap()
    bf_d = nc.dram_tensor("bfp", [4, 2, 128], F32, kind="ExternalInput").ap()
    br_d = nc.dram_tensor("brp", [4, 2, 128], F32, kind="ExternalInput").ap()
    gm_d = nc.dram_tensor("gm", [C], F32, kind="ExternalInput").ap()
    bt_d = nc.dram_tensor("bt", [C], F32, kind="ExternalInput").ap()
    cb_d = nc.dram_tensor("cb", [C], F32, kind="ExternalInput").ap()
    out_d = nc.dram_tensor("o", [C, NF], F32, kind="ExternalOutput").ap()
    scA_d = nc.dram_tensor("scA", [NLOC, 128], F32).ap()
    scB_d = nc.dram_tensor("scB", [NLOC, 128], F32).ap()

    with tile.TileContext(nc) as tc:
        with tc.tile_pool(name="const", bufs=1) as cp:
            # ---- weights / biases resident in SBUF ----
            w0_t = cp.tile([128, 2 * 4 * 512], BF16)
            w0_v = w0_t[:].rearrange("p (d kp m) -> p d kp m", d=2, kp=4)
            nc.sync.dma_start(w0_v, w0_d.rearrange("d kp p m -> p d kp m"))
            wi_t = cp.tile([128, 3 * 2 * 2 * 512], BF16)
            wi_v = wi_t[:].rearrange("p (i d ct m) -> p i d ct m", i=3, d=2, ct=2)
            nc.sync.dma_start(wi_v, wi_d.rearrange("i d ct p m -> p i d ct m"))
            cw_t = cp.tile([128, 2 * 8 * 64], BF16)
            cw_v = cw_t[:].rearrange("p (ct k m) -> p ct k m", ct=2, k=8)
            nc.sync.dma_start(cw_v, cw_d.rearrange("ct k p m -> p ct k m"))
            bfp_t = cp.tile([128, 8], F32)
            nc.sync.dma_start(bfp_t[:].rearrange("p (i d) -> p i d", i=4), bf_d.rearrange("i d p -> p i d"))
            brp_t = cp.tile([128, 8], F32)
            nc.sync.dma_start(brp_t[:].rearrange("p (i d) -> p i d", i=4), br_d.rearrange("i d p -> p i d"))
            gm1_t = cp.tile([1, C], F32)
            nc.sync.dma_start(gm1_t[:], gm_d.rearrange("(a c) -> a c", a=1))
            bt_t = cp.tile([C, 1], F32)
            nc.sync.dma_start(bt_t[:], bt_d.rearrange("(c a) -> c a", a=1))
            cb_t = cp.tile([C, 1], F32)
            nc.sync.dma_start(cb_t[:], cb_d.rearrange("(c a) -> c a", a=1))

            # ---- long-lived activations ----
            xn_t = cp.tile([C, NF], F32)          # normed input, fp32 (residual)
            xn2_t = cp.tile([128, XCOLS], DT_H)   # [xn ; xn shifted by 1] bf16
            h_t = [cp.tile([128, XCOLS], DT_H, name=f"h{i}") for i in range(4)]  # ping-pong pairs

            nc.gpsimd.memset(xn2_t[:, NF:XCOLS], 0.0)
            nc.gpsimd.memset(xn2_t[64:128, NF - 1:NF], 0.0)
            # zero h tiles once: pads stay zero through all layers (the
            # highway writes only valid columns)
            for i in range(4):
                nc.gpsimd.memset(h_t[i][:], 0.0)

            # ================= channel norm =================
            scA_f = scA_d.rearrange("n f -> (n f)")
            scB_f = scB_d.rearrange("n f -> (n f)")
            with tc.tile_pool(name="normu", bufs=1) as np_:
                u_cn = np_.tile([C, NF], F32)
                nc.sync.dma_start(u_cn[:], u_d.rearrange("c n f -> c (n f)"))
                with (
                    tc.tile_pool(name="normn", bufs=1) as nnp,
                    tc.tile_pool(name="norms", bufs=1) as nsp,
                ):
                    u_nn = nnp.tile([NLOC, C * 128], F32)
                    nc.sync.dma_start(
                        u_nn[:].rearrange("n (c f) -> n c f", f=128),
                        u_d.rearrange("c n f -> n c f"),
                    )
                    # stats over c (innermost of (n, f, c) view)
                    mu_t = nsp.tile([NLOC, 128], F32)
                    s2_t = nsp.tile([NLOC, 128], F32)
                    tmp_t = nsp.tile([NLOC, 128], F32)
                    A_t = nsp.tile([NLOC, 128], F32)
                    B_t = nsp.tile([NLOC, 128], F32)
                    un_v = u_nn[:].rearrange("n (c f) -> n f c", f=128)
                    nc.vector.tensor_reduce(mu_t[:], un_v, axis=mybir.AxisListType.X, op=OP.add)
                    zb_t = nsp.tile([NLOC, 1], F32)
                    nc.vector.memset(zb_t[:], 0.0)
                    sq_t = nsp.tile([NLOC, 16 * C], F32)
                    sq_v = sq_t[:].rearrange("n (f c) -> n f c", f=16)
                    for fc in range(8):
                        fsl = slice(fc * 16, (fc + 1) * 16)
                        nc.scalar.activation(
                            sq_v, un_v[:, fsl, :], AF.Square, bias=zb_t[:, 0:1])
                        nc.vector.tensor_reduce(s2_t[:, fsl], sq_v,
                                                axis=mybir.AxisListType.X, op=OP.add)
                    nc.vector.tensor_scalar_mul(mu_t[:], mu_t[:], 1.0 / C)
                    nc.vector.tensor_scalar_mul(s2_t[:], s2_t[:], 1.0 / C)
                    nc.vector.tensor_mul(tmp_t[:], mu_t[:], mu_t[:])
                    nc.vector.tensor_sub(s2_t[:], s2_t[:], tmp_t[:])  # var
                    eps_t = nsp.tile([NLOC, 1], F32)
                    nc.vector.memset(eps_t[:], EPS)
                    nc.scalar.activation(tmp_t[:], s2_t[:], AF.Sqrt, bias=eps_t[:, 0:1])
                    nc.vector.reciprocal(A_t[:], tmp_t[:])            # rstd
                    nc.vector.scalar_tensor_tensor(
                        B_t[:], mu_t[:], -1.0, A_t[:], op0=OP.mult, op1=OP.mult
                    )
                    # stats to DRAM, re-read as flat rows per chunk
                    nc.sync.dma_start(scA_d, A_t[:])
                    nc.sync.dma_start(scB_d, B_t[:])

                # broadcast along c with gamma folded in, then apply
                CH = 1024
                with (
                    tc.tile_pool(name="normab", bufs=4) as nab,
                    tc.tile_pool(name="normps", bufs=2, space="PSUM") as npp,
                ):
                    for ch in range(NF // CH):
                        ag = npp.tile([C, CH], F32, tag="ag")
                        bg = npp.tile([C, CH], F32, tag="bg")
                        for h2 in range(CH // 512):
                            lo = ch * CH + h2 * 512
                            a1 = nab.tile([1, 512], F32, tag="a1")
                            b1 = nab.tile([1, 512], F32, tag="b1")
                            nc.sync.dma_start(
                                a1[:], scA_f[lo:lo + 512].rearrange("(a x) -> a x", a=1))
                            nc.sync.dma_start(
                                b1[:], scB_f[lo:lo + 512].rearrange("(a x) -> a x", a=1))
                            nc.tensor.matmul(ag[:, h2 * 512:(h2 + 1) * 512], gm1_t[:],
                                             a1[:], start=True, stop=True)
                            nc.tensor.matmul(bg[:, h2 * 512:(h2 + 1) * 512], gm1_t[:],
                                             b1[:], start=True, stop=True)
                        sl = slice(ch * CH, (ch + 1) * CH)
                        nc.vector.tensor_mul(xn_t[:, sl], u_cn[:, sl], ag[:])
                        nc.vector.scalar_tensor_tensor(
                            xn_t[:, sl], xn_t[:, sl], bt_t[:, 0:1], bg[:],
                            op0=OP.add, op1=OP.add,
                        )
                        # bf16 copies into xn2 (rows 0:64 plain, 64:128 shifted by 1)
                        nc.scalar.copy(xn2_t[0:64, sl], xn_t[:, sl])
                        nc.scalar.copy(
                            xn2_t[64:128, ch * CH:(ch + 1) * CH - 1],
                            xn_t[:, ch * CH + 1:(ch + 1) * CH],
                        )
                    # chunk-boundary columns of the shifted copy (read the
                    # first col of the next chunk, so emitted after the loop)
                    xn_bv = xn_t[:].rearrange("p (a b) -> p a b", b=CH)
                    x2_bv = xn2_t[64:128, 0:NF].rearrange("p (a b) -> p a b", b=CH)
                    nc.scalar.copy(
                        x2_bv[:, 0:NF // CH - 1, CH - 1:CH],
                        xn_bv[:, 1:NF // CH, 0:1],
                    )

            # ================= SRU layers =================
            sig = AF.Sigmoid
            with (
                tc.tile_pool(name="gates", bufs=2) as gp,
                tc.tile_pool(name="lps", bufs=1, space="PSUM") as pp,
            ):
                for li in range(4):
                    if li == 0:
                        hin = None
                        nct = 4
                    else:
                        hin = [h_t[2 * ((li - 1) % 2)], h_t[2 * ((li - 1) % 2) + 1]]
                        nct = 2
                    hout = [h_t[2 * (li % 2)], h_t[2 * (li % 2) + 1]]
                    ooff = 8 if li == 3 else 0
                    for d in range(2):
                        bcol = bfp_t[:, 2 * li + d:2 * li + d + 1]
                        rcol = brp_t[:, 2 * li + d:2 * li + d + 1]
                        for half in range(2):
                            f_t = gp.tile([128, NF // 2], DT_H, tag="f")
                            b_t = gp.tile([128, NF // 2], DT_H, tag="b")
                            r_t = gp.tile([128, NF // 2], DT_H, tag="r")
                            w_t = gp.tile([128, NF // 2], DT_H, tag="w")
                            f_v = f_t[:].rearrange("p (n l) -> p n l", l=128)
                            b_v = b_t[:].rearrange("p (n l) -> p n l", l=128)
                            for s4 in range(NSPAN // 2):
                                span = half * (NSPAN // 2) + s4
                                zf_ps = pp.tile([128, 2 * SPAN], F32, name="zf", tag="zf")
                                rh_ps = pp.tile([128, 2 * SPAN], F32, name="rh", tag="rh")
                                pst = [zf_ps[:, 0:SPAN], zf_ps[:, SPAN:2 * SPAN],
                                       rh_ps[:, 0:SPAN], rh_ps[:, SPAN:2 * SPAN]]
                                for o in range(4):
                                    for h2 in range(SPAN // 512):
                                        osl = pst[o][:, h2 * 512:(h2 + 1) * 512]  # noqa
                                        base = span * SPAN + h2 * 512
                                        for ct in range(nct):
                                            if li == 0:
                                                rhs = xn2_t[:, base + 2 * ct:base + 2 * ct + 512]
                                                lhsT = w0_v[:, d, ct, o * 128:(o + 1) * 128]
                                            else:
                                                rhs = hin[ct][:, base:base + 512]
                                                lhsT = wi_v[:, li - 1, d, ct, o * 128:(o + 1) * 128]
                                            nc.tensor.matmul(
                                                osl, lhsT, rhs,
                                                start=(ct == 0), stop=(ct == nct - 1),
                                            )
                                # evacuate span, full 128-blocks. d=1 stores each
                                # block reversed (pads land at l' in [0,7)).
                                ssl = slice(s4 * SPAN, (s4 + 1) * SPAN)
                                if d == 0:
                                    srcs = list(pst)
                                else:
                                    srcs = [pq.rearrange("p (n l) -> p n l", l=128)[:, :, ::-1]
                                            for pq in pst]
                                # g = 1-f = sigmoid(-(x+bf)); bfp holds -bf
                                nc.scalar.activation(f_t[:, ssl], srcs[1], sig,
                                                     bias=bcol, scale=-1.0)
                                nc.scalar.activation(r_t[:, ssl], srcs[2], sig, bias=rcol)
                                nc.scalar.copy(b_t[:, ssl], srcs[0])   # z
                                nc.scalar.copy(w_t[:, ssl], srcs[3])   # hp
                                # b'' = g*z (in place over z; must read g before
                                # the 1-g pass below overwrites it)
                                nc.vector.tensor_mul(b_t[:, ssl], f_t[:, ssl], b_t[:, ssl])
                            # f = 1 - g  (tensor_scalar, 4x mode)
                            nc.vector.tensor_scalar(f_t[:], f_t[:], -1.0, 1.0,
                                                    op0=OP.mult, op1=OP.add)
                            # pads reset the scan carry between sequences
                            pads = slice(121, 128) if d == 0 else slice(0, 7)
                            nc.gpsimd.memset(f_v[:, :, pads], 0.0)
                            nc.gpsimd.memset(b_v[:, :, pads], 0.0)
                            # c = f*c + (1-f)*z
                            nc.vector.tensor_tensor_scan(
                                b_t[:], f_t[:], b_t[:], 0.0,
                                op0=OP.mult, op1=OP.add,
                            )
                            # highway: out = r*(cs-hp) + hp
                            nc.vector.tensor_sub(f_t[:], b_t[:], w_t[:])
                            nc.vector.tensor_mul(r_t[:], r_t[:], f_t[:])
                            hov = hout[d][:, ooff:ooff + NF].rearrange(
                                "p (n l) -> p n l", l=128
                            )
                            dst = hov[:, half * 32:half * 32 + 32, :]
                            if d == 1:
                                dst = dst[:, :, ::-1]
                            r_v = r_t[:].rearrange("p (n l) -> p n l", l=128)
                            w_v = w_t[:].rearrange("p (n l) -> p n l", l=128)
                            nc.vector.tensor_add(dst, r_v[:, :, :], w_v[:, :, :])

            # ================= transposed conv + residual =================
            h4 = [h_t[2], h_t[3]]  # layer 3 writes pair B at offset 8
            for t4 in h4:
                v = t4[:, 0:NF].rearrange("p (n l) -> p n l", l=128)
                nc.gpsimd.memset(t4[:, 0:8], 0.0)
                nc.gpsimd.memset(v[:, 1:33, 1:8], 0.0)
                nc.gpsimd.memset(v[:, 33:64, 1:8], 0.0)
                nc.gpsimd.memset(t4[:, NF + 1:XCOLS], 0.0)
            with (
                tc.tile_pool(name="cvp", bufs=4, space="PSUM") as cvp,
                tc.tile_pool(name="osp", bufs=2) as osp,
            ):
                for span in range(NSPAN):
                    c_ps = cvp.tile([C, SPAN], F32, tag="c")
                    for h2 in range(SPAN // 512):
                        osl = c_ps[:, h2 * 512:(h2 + 1) * 512]
                        base = span * SPAN + h2 * 512
                        mm = 0
                        for ct in range(2):
                            for k in range(8):
                                rhs = h4[ct][:, 8 - k + base:8 - k + base + 512]
                                nc.tensor.matmul(
                                    osl, cw_v[:, ct, k, :], rhs,
                                    start=(mm == 0), stop=(mm == 15),
                                )
                                mm += 1
                    o_t = osp.tile([C, SPAN], F32, tag="o")
                    sl = slice(span * SPAN, (span + 1) * SPAN)
                    nc.vector.scalar_tensor_tensor(
                        o_t[:], c_ps[:], cb_t[:, 0:1], xn_t[:, sl],
                        op0=OP.add, op1=OP.add,
                    )
                    nc.sync.dma_start(out_d[:, sl], o_t[:])

    nc.compile()
    return nc


def _prep_weights(W0, Ws, convW):
    w0r = W0.reshape(C, K, 2, 4 * H)
    w0p = np.zeros((2, 4, 128, 512), np.float32)
    for d in range(2):
        for kp in range(4):
            w0p[d, kp, 0:64] = w0r[:, 2 * kp, d]
            w0p[d, kp, 64:128] = w0r[:, 2 * kp + 1, d]
    wip = np.zeros((3, 2, 2, 128, 512), np.float32)
    for i in range(3):
        for d in range(2):
            for ct in range(2):
                wip[i, d, ct] = Ws[i][ct * 128:(ct + 1) * 128, d]
    cwp = np.zeros((2, 8, 128, C), np.float32)
    for ct in range(2):
        for k in range(8):
            cwp[ct, k] = convW[ct * 128:(ct + 1) * 128, :, k]
    bf16 = ml_dtypes.bfloat16
    return w0p.astype(bf16), wip.astype(bf16), cwp.astype(bf16)


def kernel(**inputs):
    inputs = {k: np.asarray(v) for k, v in inputs.items()}
    x = inputs["x"].astype(np.float32)
    xs = np.ascontiguousarray(
        x.transpose(0, 2, 1, 3).reshape(B * T, C, F_)
    )  # (512, C, F)

    w0p, wip, cwp = _prep_weights(
        inputs["W0"].astype(np.float32),
        [inputs[f"W{i}"].astype(np.float32) for i in (1, 2, 3)],
        inputs["convW"].astype(np.float32),
    )
    bfp = -np.stack([inputs[f"bf{i}"] for i in range(4)]).astype(np.float32)
    brp = np.stack([inputs[f"br{i}"] for i in range(4)]).astype(np.float32)
    gm = inputs["gamma"].reshape(C).astype(np.float32)
    bt = inputs["beta"].reshape(C).astype(np.float32)
    cb = inputs["convb"].reshape(C).astype(np.float32)

    if "nc" not in _CACHE:
        _CACHE["nc"] = _build()
    nc = _CACHE["nc"]

    shared = {"w0p": w0p, "wip": wip, "cwp": cwp, "bfp": bfp, "brp": brp,
              "gm": gm, "bt": bt, "cb": cb}
    in_maps = []
    for core in range(NCORES):
        u = np.ascontiguousarray(
            xs[core * NLOC:(core + 1) * NLOC].transpose(1, 0, 2)
        )  # (C, NLOC, F)
        in_maps.append({"u": u, **shared})

    trace = bool(os.environ.get("KBENCH_TRACE"))
    res = bass_utils.run_bass_kernel_spmd(
        nc, in_maps, list(range(NCORES)), trace=trace,
        tmpdir=os.environ.get("KBENCH_TMPDIR"),
    )
    _CACHE["last_result"] = res

    full = np.concatenate(
        [res.results[i]["o"].reshape(C, NLOC, F_) for i in range(NCORES)], axis=1
    )  # (C, 512, F)
    out = full.transpose(1, 0, 2).reshape(B, T, C, F_).transpose(0, 2, 1, 3)
    return np.ascontiguousarray(out.astype(np.float32))


# revision 19
# speedup vs baseline: 1.2882x; 1.0411x over previous
"""Trainium2 Bass kernel for the DPRNN block (channel-norm -> unfold ->
4x bidirectional SRU -> conv-transpose -> residual).

Sharding: data-parallel over the B*T=512 sequences; 64 sequences per core.
All weights replicated. Each core runs the full pipeline on its shard.

Layout (per core): sequences live in 128-column blocks (121 valid SRU steps
+ 7 pad columns). Pads carry f=0, b=0 through the scan so a single
tensor_tensor_scan over the whole free dim handles all sequences.
"""
import os
import numpy as np
import ml_dtypes

import concourse.bass as bass
import concourse.mybir as mybir
import concourse.tile as tile
from concourse import bacc
from concourse import bass_utils

F32 = mybir.dt.float32
BF16 = mybir.dt.bfloat16

B, C, T, F_ = 4, 64, 128, 128
H, K = 128, 8
L = F_ - K + 1            # 121
EPS = 1e-8
NCORES = 8
NLOC = (B * T) // NCORES  # 64 sequences per core
NF = NLOC * 128           # 8192
XCOLS = NF + 8            # xn2 / h tiles carry 8 extra cols for shifted reads

DT_H = BF16       # h / xn2 / gate dtype (matmul inputs)
SPAN = 1024       # psum evacuation span (8 seqs)
NSPAN = NF // SPAN

_CACHE = {}


def _build():
    nc = bacc.Bacc("TRN2", target_bir_lowering=False, debug=False)
    AF = mybir.ActivationFunctionType
    OP = mybir.AluOpType

    # ---------------- DRAM tensors ----------------
    u_d = nc.dram_tensor("u", [C, NLOC, F_], F32, kind="ExternalInput").ap()
    un_d = nc.dram_tensor("un", [NLOC, C, F_], F32, kind="ExternalInput").ap()
    w0_d = nc.dram_tensor("w0p", [2, 4, 128, 512], BF16, kind="ExternalInput").ap()
    wi_d = nc.dram_tensor("wip", [3, 2, 2, 128, 512], BF16, kind="ExternalInput").ap()
    cw_d = nc.dram_tensor("cwp", [2, 8, 128, 64], BF16, kind="ExternalInput").ap()
    bf_d = nc.dram_tensor("bfp", [4, 2, 128], F32, kind="ExternalInput").ap()
    br_d = nc.dram_tensor("brp", [4, 2, 128], F32, kind="ExternalInput").ap()
    gm_d = nc.dram_tensor("gm", [C], F32, kind="ExternalInput").ap()
    bt_d = nc.dram_tensor("bt", [C], F32, kind="ExternalInput").ap()
    cb_d = nc.dram_tensor("cb", [C], F32, kind="ExternalInput").ap()
    out_d = nc.dram_tensor("o", [C, NF], F32, kind="ExternalOutput").ap()
    scA_d = nc.dram_tensor("scA", [NLOC, 128], F32).ap()
    scB_d = nc.dram_tensor("scB", [NLOC, 128], F32).ap()

    with tile.TileContext(nc) as tc:
        with tc.tile_pool(name="const", bufs=1) as cp:
            # ---- weights / biases resident in SBUF ----
            w0_t = cp.tile([128, 2 * 4 * 512], BF16)
            w0_v = w0_t[:].rearrange("p (d kp m) -> p d kp m", d=2, kp=4)
            nc.sync.dma_start(w0_v, w0_d.rearrange("d kp p m -> p d kp m"))
            wi_t = cp.tile([128, 3 * 2 * 2 * 512], BF16)
            wi_v = wi_t[:].rearrange("p (i d ct m) -> p i d ct m", i=3, d=2, ct=2)
            nc.sync.dma_start(wi_v, wi_d.rearrange("i d ct p m -> p i d ct m"))
            cw_t = cp.tile([128, 2 * 8 * 64], BF16)
            cw_v = cw_t[:].rearrange("p (ct k m) -> p ct k m", ct=2, k=8)
            nc.sync.dma_start(cw_v, cw_d.rearrange("ct k p m -> p ct k m"))
            bfp_t = cp.tile([128, 8], F32)
            nc.sync.dma_start(bfp_t[:].rearrange("p (i d) -> p i d", i=4), bf_d.rearrange("i d p -> p i d"))
            brp_t = cp.tile([128, 8], F32)
            nc.sync.dma_start(brp_t[:].rearrange("p (i d) -> p i d", i=4), br_d.rearrange("i d p -> p i d"))
            gm1_t = cp.tile([1, C], F32)
            nc.sync.dma_start(gm1_t[:], gm_d.rearrange("(a c) -> a c", a=1))
            bt_t = cp.tile([C, 1], F32)
            nc.sync.dma_start(bt_t[:], bt_d.rearrange("(c a) -> c a", a=1))
            cb_t = cp.tile([C, 1], F32)
            nc.sync.dma_start(cb_t[:], cb_d.rearrange("(c a) -> c a", a=1))

            # ---- long-lived activations ----
            xn_t = cp.tile([C, NF], F32)          # normed input, fp32 (residual)
            xn2_t = cp.tile([128, XCOLS], DT_H)   # [xn ; xn shifted by 1] bf16
            h_t = [cp.tile([128, XCOLS], DT_H, name=f"h{i}") for i in range(4)]  # ping-pong pairs

            nc.gpsimd.memset(xn2_t[:, NF:XCOLS], 0.0)
            nc.gpsimd.memset(xn2_t[64:128, NF - 1:NF], 0.0)
            # zero h tiles once: pads stay zero through all layers (the
            # highway writes only valid columns)
            for i in range(4):
                nc.gpsimd.memset(h_t[i][:], 0.0)

            # ================= channel norm =================
            scA_f = scA_d.rearrange("n f -> (n f)")
            scB_f = scB_d.rearrange("n f -> (n f)")
            with tc.tile_pool(name="normu", bufs=1) as np_:
                u_cn = np_.tile([C, NF], F32)
                nc.sync.dma_start(u_cn[:], u_d.rearrange("c n f -> c (n f)"))
                with (
                    tc.tile_pool(name="normn", bufs=1) as nnp,
                    tc.tile_pool(name="norms", bufs=1) as nsp,
                ):
                    u_nn = nnp.tile([NLOC, C * 128], F32)
                    nc.scalar.dma_start(u_nn[:], un_d.rearrange("n c f -> n (c f)"))
                    # stats over c (innermost of (n, f, c) view)
                    mu_t = nsp.tile([NLOC, 128], F32)
                    s2_t = nsp.tile([NLOC, 128], F32)
                    tmp_t = nsp.tile([NLOC, 128], F32)
                    A_t = nsp.tile([NLOC, 128], F32)
                    B_t = nsp.tile([NLOC, 128], F32)
                    un_v = u_nn[:].rearrange("n (c f) -> n f c", f=128)
                    nc.vector.tensor_reduce(mu_t[:], un_v, axis=mybir.AxisListType.X, op=OP.add)
                    zb_t = nsp.tile([NLOC, 1], F32)
                    nc.vector.memset(zb_t[:], 0.0)
                    sq_t = nsp.tile([NLOC, 16 * C], F32)
                    sq_v = sq_t[:].rearrange("n (f c) -> n f c", f=16)
                    for fc in range(8):
                        fsl = slice(fc * 16, (fc + 1) * 16)
                        nc.scalar.activation(
                            sq_v, un_v[:, fsl, :], AF.Square, bias=zb_t[:, 0:1])
                        nc.vector.tensor_reduce(s2_t[:, fsl], sq_v,
                                                axis=mybir.AxisListType.X, op=OP.add)
                    nc.vector.tensor_scalar_mul(mu_t[:], mu_t[:], 1.0 / C)
                    nc.vector.tensor_scalar_mul(s2_t[:], s2_t[:], 1.0 / C)
                    nc.vector.tensor_mul(tmp_t[:], mu_t[:], mu_t[:])
                    nc.vector.tensor_sub(s2_t[:], s2_t[:], tmp_t[:])  # var
                    eps_t = nsp.tile([NLOC, 1], F32)
                    nc.vector.memset(eps_t[:], EPS)
                    nc.scalar.activation(tmp_t[:], s2_t[:], AF.Sqrt, bias=eps_t[:, 0:1])
                    nc.vector.reciprocal(A_t[:], tmp_t[:])            # rstd
                    nc.vector.scalar_tensor_tensor(
                        B_t[:], mu_t[:], -1.0, A_t[:], op0=OP.mult, op1=OP.mult
                    )
                    # stats to DRAM, re-read as flat rows per chunk
                    nc.sync.dma_start(scA_d, A_t[:])
                    nc.sync.dma_start(scB_d, B_t[:])

                # broadcast along c with gamma folded in, then apply
                CH = 1024
                with (
                    tc.tile_pool(name="normab", bufs=4) as nab,
                    tc.tile_pool(name="normps", bufs=2, space="PSUM") as npp,
                ):
                    for ch in range(NF // CH):
                        ag = npp.tile([C, CH], F32, tag="ag")
                        bg = npp.tile([C, CH], F32, tag="bg")
                        for h2 in range(CH // 512):
                            lo = ch * CH + h2 * 512
                            a1 = nab.tile([1, 512], F32, tag="a1")
                            b1 = nab.tile([1, 512], F32, tag="b1")
                            nc.sync.dma_start(
                                a1[:], scA_f[lo:lo + 512].rearrange("(a x) -> a x", a=1))
                            nc.sync.dma_start(
                                b1[:], scB_f[lo:lo + 512].rearrange("(a x) -> a x", a=1))
                            nc.tensor.matmul(ag[:, h2 * 512:(h2 + 1) * 512], gm1_t[:],
                                             a1[:], start=True, stop=True)
                            nc.tensor.matmul(bg[:, h2 * 512:(h2 + 1) * 512], gm1_t[:],
                                             b1[:], start=True, stop=True)
                        sl = slice(ch * CH, (ch + 1) * CH)
                        nc.vector.tensor_mul(xn_t[:, sl], u_cn[:, sl], ag[:])
                        nc.vector.scalar_tensor_tensor(
                            xn_t[:, sl], xn_t[:, sl], bt_t[:, 0:1], bg[:],
                            op0=OP.add, op1=OP.add,
                        )
                        # bf16 copies into xn2 (rows 0:64 plain, 64:128 shifted by 1)
                        nc.scalar.copy(xn2_t[0:64, sl], xn_t[:, sl])
                        nc.scalar.copy(
                            xn2_t[64:128, ch * CH:(ch + 1) * CH - 1],
                            xn_t[:, ch * CH + 1:(ch + 1) * CH],
                        )
                    # chunk-boundary columns of the shifted copy (read the
                    # first col of the next chunk, so emitted after the loop)
                    xn_bv = xn_t[:].rearrange("p (a b) -> p a b", b=CH)
                    x2_bv = xn2_t[64:128, 0:NF].rearrange("p (a b) -> p a b", b=CH)
                    nc.scalar.copy(
                        x2_bv[:, 0:NF // CH - 1, CH - 1:CH],
                        xn_bv[:, 1:NF // CH, 0:1],
                    )

            # ================= SRU layers =================
            sig = AF.Sigmoid
            with (
                tc.tile_pool(name="gates", bufs=2) as gp,
                tc.tile_pool(name="lps", bufs=1, space="PSUM") as pp,
            ):
                for li in range(4):
                    if li == 0:
                        hin = None
                        nct = 4
                    else:
                        hin = [h_t[2 * ((li - 1) % 2)], h_t[2 * ((li - 1) % 2) + 1]]
                        nct = 2
                    hout = [h_t[2 * (li % 2)], h_t[2 * (li % 2) + 1]]
                    ooff = 8 if li == 3 else 0
                    for d in range(2):
                        bcol = bfp_t[:, 2 * li + d:2 * li + d + 1]
                        rcol = brp_t[:, 2 * li + d:2 * li + d + 1]
                        for half in range(2):
                            f_t = gp.tile([128, NF // 2], DT_H, tag="f")
                            b_t = gp.tile([128, NF // 2], DT_H, tag="b")
                            r_t = gp.tile([128, NF // 2], DT_H, tag="r")
                            w_t = gp.tile([128, NF // 2], DT_H, tag="w")
                            f_v = f_t[:].rearrange("p (n l) -> p n l", l=128)
                            b_v = b_t[:].rearrange("p (n l) -> p n l", l=128)
                            for s4 in range(NSPAN // 2):
                                span = half * (NSPAN // 2) + s4
                                zf_ps = pp.tile([128, 2 * SPAN], F32, name="zf", tag="zf")
                                rh_ps = pp.tile([128, 2 * SPAN], F32, name="rh", tag="rh")
                                pst = [zf_ps[:, 0:SPAN], zf_ps[:, SPAN:2 * SPAN],
                                       rh_ps[:, 0:SPAN], rh_ps[:, SPAN:2 * SPAN]]
                                for o in range(4):
                                    for h2 in range(SPAN // 512):
                                        osl = pst[o][:, h2 * 512:(h2 + 1) * 512]  # noqa
                                        base = span * SPAN + h2 * 512
                                        for ct in range(nct):
                                            if li == 0:
                                                rhs = xn2_t[:, base + 2 * ct:base + 2 * ct + 512]
                                                lhsT = w0_v[:, d, ct, o * 128:(o + 1) * 128]
                                            else:
                                                rhs = hin[ct][:, base:base + 512]
                                                lhsT = wi_v[:, li - 1, d, ct, o * 128:(o + 1) * 128]
                                            nc.tensor.matmul(
                                                osl, lhsT, rhs,
                                                start=(ct == 0), stop=(ct == nct - 1),
                                            )
                                # evacuate span, full 128-blocks. d=1 stores each
                                # block reversed (pads land at l' in [0,7)).
                                ssl = slice(s4 * SPAN, (s4 + 1) * SPAN)
                                if d == 0:
                                    srcs = list(pst)
                                else:
                                    srcs = [pq.rearrange("p (n l) -> p n l", l=128)[:, :, ::-1]
                                            for pq in pst]
                                # g = 1-f = sigmoid(-(x+bf)); bfp holds -bf
                                nc.scalar.activation(f_t[:, ssl], srcs[1], sig,
                                                     bias=bcol, scale=-1.0)
                                nc.scalar.activation(r_t[:, ssl], srcs[2], sig, bias=rcol)
                                nc.scalar.copy(b_t[:, ssl], srcs[0])   # z
                                nc.scalar.copy(w_t[:, ssl], srcs[3])   # hp
                            # b'' = g*z (in place over z; must read g before
                            # the 1-g pass below overwrites it)
                            nc.vector.tensor_mul(b_t[:], f_t[:], b_t[:])
                            # f = 1 - g  (tensor_scalar, 4x mode)
                            nc.vector.tensor_scalar(f_t[:], f_t[:], -1.0, 1.0,
                                                    op0=OP.mult, op1=OP.add)
                            # pads reset the scan carry between sequences
                            pads = slice(121, 128) if d == 0 else slice(0, 7)
                            nc.gpsimd.memset(f_v[:, :, pads], 0.0)
                            nc.gpsimd.memset(b_v[:, :, pads], 0.0)
                            # c = f*c + (1-f)*z
                            nc.vector.tensor_tensor_scan(
                                b_t[:], f_t[:], b_t[:], 0.0,
                                op0=OP.mult, op1=OP.add,
                            )
                            # highway: out = r*(cs-hp) + hp
                            nc.vector.tensor_sub(f_t[:], b_t[:], w_t[:])
                            nc.vector.tensor_mul(r_t[:], r_t[:], f_t[:])
                            hov = hout[d][:, ooff:ooff + NF].rearrange(
                                "p (n l) -> p n l", l=128
                            )
                            dst = hov[:, half * 32:half * 32 + 32, :]
                            if d == 1:
                                dst = dst[:, :, ::-1]
                            r_v = r_t[:].rearrange("p (n l) -> p n l", l=128)
                            w_v = w_t[:].rearrange("p (n l) -> p n l", l=128)
                            nc.vector.tensor_add(dst, r_v[:, :, :], w_v[:, :, :])

            # ================= transposed conv + residual =================
            h4 = [h_t[2], h_t[3]]  # layer 3 writes pair B at offset 8
            for t4 in h4:
                v = t4[:, 0:NF].rearrange("p (n l) -> p n l", l=128)
                nc.gpsimd.memset(t4[:, 0:8], 0.0)
                nc.gpsimd.memset(v[:, 1:33, 1:8], 0.0)
                nc.gpsimd.memset(v[:, 33:64, 1:8], 0.0)
                nc.gpsimd.memset(t4[:, NF + 1:XCOLS], 0.0)
            with (
                tc.tile_pool(name="cvp", bufs=4, space="PSUM") as cvp,
                tc.tile_pool(name="osp", bufs=2) as osp,
            ):
                for span in range(NSPAN):
                    c_ps = cvp.tile([C, SPAN], F32, tag="c")
                    for h2 in range(SPAN // 512):
                        osl = c_ps[:, h2 * 512:(h2 + 1) * 512]
                        base = span * SPAN + h2 * 512
                        mm = 0
                        for ct in range(2):
                            for k in range(8):
                                rhs = h4[ct][:, 8 - k + base:8 - k + base + 512]
                                nc.tensor.matmul(
                                    osl, cw_v[:, ct, k, :], rhs,
                                    start=(mm == 0), stop=(mm == 15),
                                )
                                mm += 1
                    o_t = osp.tile([C, SPAN], F32, tag="o")
                    sl = slice(span * SPAN, (span + 1) * SPAN)
                    nc.vector.scalar_tensor_tensor(
                        o_t[:], c_ps[:], cb_t[:, 0:1], xn_t[:, sl],
                        op0=OP.add, op1=OP.add,
                    )
                    nc.sync.dma_start(out_d[:, sl], o_t[:])

    nc.compile()
    return nc


def _prep_weights(W0, Ws, convW):
    w0r = W0.reshape(C, K, 2, 4 * H)
    w0p = np.zeros((2, 4, 128, 512), np.float32)
    for d in range(2):
        for kp in range(4):
            w0p[d, kp, 0:64] = w0r[:, 2 * kp, d]
            w0p[d, kp, 64:128] = w0r[:, 2 * kp + 1, d]
    wip = np.zeros((3, 2, 2, 128, 512), np.float32)
    for i in range(3):
        for d in range(2):
            for ct in range(2):
                wip[i, d, ct] = Ws[i][ct * 128:(ct + 1) * 128, d]
    cwp = np.zeros((2, 8, 128, C), np.float32)
    for ct in range(2):
        for k in range(8):
            cwp[ct, k] = convW[ct * 128:(ct + 1) * 128, :, k]
    bf16 = ml_dtypes.bfloat16
    return w0p.astype(bf16), wip.astype(bf16), cwp.astype(bf16)


def kernel(**inputs):
    inputs = {k: np.asarray(v) for k, v in inputs.items()}
    x = inputs["x"].astype(np.float32)
    xs = np.ascontiguousarray(
        x.transpose(0, 2, 1, 3).reshape(B * T, C, F_)
    )  # (512, C, F)

    w0p, wip, cwp = _prep_weights(
        inputs["W0"].astype(np.float32),
        [inputs[f"W{i}"].astype(np.float32) for i in (1, 2, 3)],
        inputs["convW"].astype(np.float32),
    )
    bfp = -np.stack([inputs[f"bf{i}"] for i in range(4)]).astype(np.float32)
    brp = np.stack([inputs[f"br{i}"] for i in range(4)]).astype(np.float32)
    gm = inputs["gamma"].reshape(C).astype(np.float32)
    bt = inputs["beta"].reshape(C).astype(np.float32)
    cb = inputs["convb"].reshape(C).astype(np.float32)

    if "nc" not in _CACHE:
        _CACHE["nc"] = _build()
    nc = _CACHE["nc"]

    shared = {"w0p": w0p, "wip": wip, "cwp": cwp, "bfp": bfp, "brp": brp,
              "gm": gm, "bt": bt, "cb": cb}
    in_maps = []
    for core in range(NCORES):
        sh = xs[core * NLOC:(core + 1) * NLOC]  # (NLOC, C, F)
        u = np.ascontiguousarray(sh.transpose(1, 0, 2))  # (C, NLOC, F)
        un = np.ascontiguousarray(sh)
        in_maps.append({"u": u, "un": un, **shared})

    trace = bool(os.environ.get("KBENCH_TRACE"))
    res = bass_utils.run_bass_kernel_spmd(
        nc, in_maps, list(range(NCORES)), trace=trace,
        tmpdir=os.environ.get("KBENCH_TMPDIR"),
    )
    _CACHE["last_result"] = res

    full = np.concatenate(
        [res.results[i]["o"].reshape(C, NLOC, F_) for i in range(NCORES)], axis=1
    )  # (C, 512, F)
    out = full.transpose(1, 0, 2).reshape(B, T, C, F_).transpose(0, 2, 1, 3)
    return np.ascontiguousarray(out.astype(np.float32))


# revision 20
# speedup vs baseline: 1.2955x; 1.0057x over previous
"""Trainium2 Bass kernel for the DPRNN block (channel-norm -> unfold ->
4x bidirectional SRU -> conv-transpose -> residual).

Sharding: data-parallel over the B*T=512 sequences; 64 sequences per core.
All weights replicated. Each core runs the full pipeline on its shard.

Layout (per core): sequences live in 128-column blocks (121 valid SRU steps
+ 7 pad columns). Pads carry f=0, b=0 through the scan so a single
tensor_tensor_scan over the whole free dim handles all sequences.
"""
import os
import numpy as np
import ml_dtypes

import concourse.bass as bass
import concourse.mybir as mybir
import concourse.tile as tile
from concourse import bacc
from concourse import bass_utils

F32 = mybir.dt.float32
BF16 = mybir.dt.bfloat16

B, C, T, F_ = 4, 64, 128, 128
H, K = 128, 8
L = F_ - K + 1            # 121
EPS = 1e-8
NCORES = 8
NLOC = (B * T) // NCORES  # 64 sequences per core
NF = NLOC * 128           # 8192
XCOLS = NF + 8            # xn2 / h tiles carry 8 extra cols for shifted reads

DT_H = BF16       # h / xn2 / gate dtype (matmul inputs)
SPAN = 1024       # psum evacuation span (8 seqs)
NSPAN = NF // SPAN

_CACHE = {}


def _build():
    nc = bacc.Bacc("TRN2", target_bir_lowering=False, debug=False)
    AF = mybir.ActivationFunctionType
    OP = mybir.AluOpType

    # ---------------- DRAM tensors ----------------
    u_d = nc.dram_tensor("u", [C, NLOC, F_], F32, kind="ExternalInput").ap()
    un_d = nc.dram_tensor("un", [NLOC, C, F_], F32, kind="ExternalInput").ap()
    w0_d = nc.dram_tensor("w0p", [2, 4, 128, 512], BF16, kind="ExternalInput").ap()
    wi_d = nc.dram_tensor("wip", [3, 2, 2, 128, 512], BF16, kind="ExternalInput").ap()
    cw_d = nc.dram_tensor("cwp", [2, 8, 128, 64], BF16, kind="ExternalInput").ap()
    bf_d = nc.dram_tensor("bfp", [4, 2, 128], F32, kind="ExternalInput").ap()
    br_d = nc.dram_tensor("brp", [4, 2, 128], F32, kind="ExternalInput").ap()
    gm_d = nc.dram_tensor("gm", [C], F32, kind="ExternalInput").ap()
    bt_d = nc.dram_tensor("bt", [C], F32, kind="ExternalInput").ap()
    cb_d = nc.dram_tensor("cb", [C], F32, kind="ExternalInput").ap()
    out_d = nc.dram_tensor("o", [C, NF], F32, kind="ExternalOutput").ap()
    scA_d = nc.dram_tensor("scA", [NLOC, 128], F32).ap()
    scB_d = nc.dram_tensor("scB", [NLOC, 128], F32).ap()

    with tile.TileContext(nc) as tc:
        with tc.tile_pool(name="const", bufs=1) as cp:
            # ---- weights / biases resident in SBUF ----
            w0_t = cp.tile([128, 2 * 4 * 512], BF16)
            w0_v = w0_t[:].rearrange("p (d kp m) -> p d kp m", d=2, kp=4)
            nc.sync.dma_start(w0_v, w0_d.rearrange("d kp p m -> p d kp m"))
            wi_t = cp.tile([128, 3 * 2 * 2 * 512], BF16)
            wi_v = wi_t[:].rearrange("p (i d ct m) -> p i d ct m", i=3, d=2, ct=2)
            nc.sync.dma_start(wi_v, wi_d.rearrange("i d ct p m -> p i d ct m"))
            cw_t = cp.tile([128, 2 * 8 * 64], BF16)
            cw_v = cw_t[:].rearrange("p (ct k m) -> p ct k m", ct=2, k=8)
            nc.sync.dma_start(cw_v, cw_d.rearrange("ct k p m -> p ct k m"))
            bfp_t = cp.tile([128, 8], F32)
            nc.sync.dma_start(bfp_t[:].rearrange("p (i d) -> p i d", i=4), bf_d.rearrange("i d p -> p i d"))
            brp_t = cp.tile([128, 8], F32)
            nc.sync.dma_start(brp_t[:].rearrange("p (i d) -> p i d", i=4), br_d.rearrange("i d p -> p i d"))
            gm1_t = cp.tile([1, C], F32)
            nc.sync.dma_start(gm1_t[:], gm_d.rearrange("(a c) -> a c", a=1))
            bt_t = cp.tile([C, 1], F32)
            nc.sync.dma_start(bt_t[:], bt_d.rearrange("(c a) -> c a", a=1))
            cb_t = cp.tile([C, 1], F32)
            nc.sync.dma_start(cb_t[:], cb_d.rearrange("(c a) -> c a", a=1))

            # ---- long-lived activations ----
            xn_t = cp.tile([C, NF], F32)          # normed input, fp32 (residual)
            xn2_t = cp.tile([128, XCOLS], DT_H)   # [xn ; xn shifted by 1] bf16
            h_t = [cp.tile([128, XCOLS], DT_H, name=f"h{i}") for i in range(4)]  # ping-pong pairs

            nc.gpsimd.memset(xn2_t[:, NF:XCOLS], 0.0)
            nc.gpsimd.memset(xn2_t[64:128, NF - 1:NF], 0.0)
            # zero h tiles once: pads stay zero through all layers (the
            # highway writes only valid columns)
            for i in range(4):
                nc.gpsimd.memset(h_t[i][:], 0.0)

            # ================= channel norm =================
            scA_f = scA_d.rearrange("n f -> (n f)")
            scB_f = scB_d.rearrange("n f -> (n f)")
            with tc.tile_pool(name="normu", bufs=1) as np_:
                u_cn = np_.tile([C, NF], F32)
                nc.sync.dma_start(u_cn[:], u_d.rearrange("c n f -> c (n f)"))
                with (
                    tc.tile_pool(name="normn", bufs=1) as nnp,
                    tc.tile_pool(name="norms", bufs=1) as nsp,
                ):
                    u_nn = nnp.tile([NLOC, C * 128], F32)
                    nc.scalar.dma_start(u_nn[:], un_d.rearrange("n c f -> n (c f)"))
                    # stats over c (innermost of (n, f, c) view)
                    mu_t = nsp.tile([NLOC, 128], F32)
                    s2_t = nsp.tile([NLOC, 128], F32)
                    tmp_t = nsp.tile([NLOC, 128], F32)
                    A_t = nsp.tile([NLOC, 128], F32)
                    B_t = nsp.tile([NLOC, 128], F32)
                    un_v = u_nn[:].rearrange("n (c f) -> n f c", f=128)
                    nc.vector.tensor_reduce(mu_t[:], un_v, axis=mybir.AxisListType.X, op=OP.add)
                    zb_t = nsp.tile([NLOC, 1], F32)
                    nc.vector.memset(zb_t[:], 0.0)
                    sq_t = nsp.tile([NLOC, 16 * C], F32)
                    sq_v = sq_t[:].rearrange("n (f c) -> n f c", f=16)
                    for fc in range(8):
                        fsl = slice(fc * 16, (fc + 1) * 16)
                        nc.scalar.activation(
                            sq_v, un_v[:, fsl, :], AF.Square, bias=zb_t[:, 0:1])
                        nc.vector.tensor_reduce(s2_t[:, fsl], sq_v,
                                                axis=mybir.AxisListType.X, op=OP.add)
                    nc.vector.tensor_scalar_mul(mu_t[:], mu_t[:], 1.0 / C)
                    nc.vector.tensor_scalar_mul(s2_t[:], s2_t[:], 1.0 / C)
                    nc.vector.tensor_mul(tmp_t[:], mu_t[:], mu_t[:])
                    nc.vector.tensor_sub(s2_t[:], s2_t[:], tmp_t[:])  # var
                    eps_t = nsp.tile([NLOC, 1], F32)
                    nc.vector.memset(eps_t[:], EPS)
                    nc.scalar.activation(tmp_t[:], s2_t[:], AF.Sqrt, bias=eps_t[:, 0:1])
                    nc.vector.reciprocal(A_t[:], tmp_t[:])            # rstd
                    nc.vector.scalar_tensor_tensor(
                        B_t[:], mu_t[:], -1.0, A_t[:], op0=OP.mult, op1=OP.mult
                    )
                    # stats to DRAM, re-read as flat rows per chunk
                    nc.sync.dma_start(scA_d, A_t[:])
                    nc.sync.dma_start(scB_d, B_t[:])

                # broadcast along c with gamma folded in, then apply
                CH = 1024
                with (
                    tc.tile_pool(name="normab", bufs=4) as nab,
                    tc.tile_pool(name="normps", bufs=2, space="PSUM") as npp,
                ):
                    for ch in range(NF // CH):
                        ag = npp.tile([C, CH], F32, tag="ag")
                        bg = npp.tile([C, CH], F32, tag="bg")
                        for h2 in range(CH // 512):
                            lo = ch * CH + h2 * 512
                            a1 = nab.tile([1, 512], F32, tag="a1")
                            b1 = nab.tile([1, 512], F32, tag="b1")
                            nc.sync.dma_start(
                                a1[:], scA_f[lo:lo + 512].rearrange("(a x) -> a x", a=1))
                            nc.sync.dma_start(
                                b1[:], scB_f[lo:lo + 512].rearrange("(a x) -> a x", a=1))
                            nc.tensor.matmul(ag[:, h2 * 512:(h2 + 1) * 512], gm1_t[:],
                                             a1[:], start=True, stop=True)
                            nc.tensor.matmul(bg[:, h2 * 512:(h2 + 1) * 512], gm1_t[:],
                                             b1[:], start=True, stop=True)
                        sl = slice(ch * CH, (ch + 1) * CH)
                        nc.vector.tensor_mul(xn_t[:, sl], u_cn[:, sl], ag[:])
                        nc.vector.scalar_tensor_tensor(
                            xn_t[:, sl], xn_t[:, sl], bt_t[:, 0:1], bg[:],
                            op0=OP.add, op1=OP.add,
                        )
                        # bf16 copies into xn2 (rows 0:64 plain, 64:128 shifted by 1)
                        nc.scalar.copy(xn2_t[0:64, sl], xn_t[:, sl])
                        nc.scalar.copy(
                            xn2_t[64:128, ch * CH:(ch + 1) * CH - 1],
                            xn_t[:, ch * CH + 1:(ch + 1) * CH],
                        )
                    # chunk-boundary columns of the shifted copy (read the
                    # first col of the next chunk, so emitted after the loop)
                    xn_bv = xn_t[:].rearrange("p (a b) -> p a b", b=CH)
                    x2_bv = xn2_t[64:128, 0:NF].rearrange("p (a b) -> p a b", b=CH)
                    nc.scalar.copy(
                        x2_bv[:, 0:NF // CH - 1, CH - 1:CH],
                        xn_bv[:, 1:NF // CH, 0:1],
                    )

            # ================= SRU layers =================
            sig = AF.Sigmoid
            with (
                tc.tile_pool(name="gates", bufs=2) as gp,
                tc.tile_pool(name="lps", bufs=1, space="PSUM") as pp,
            ):
                for li in range(4):
                    if li == 0:
                        hin = None
                        nct = 4
                    else:
                        hin = [h_t[2 * ((li - 1) % 2)], h_t[2 * ((li - 1) % 2) + 1]]
                        nct = 2
                    hout = [h_t[2 * (li % 2)], h_t[2 * (li % 2) + 1]]
                    ooff = 8 if li == 3 else 0
                    for half in range(2):
                        for d in range(2):
                            bcol = bfp_t[:, 2 * li + d:2 * li + d + 1]
                            rcol = brp_t[:, 2 * li + d:2 * li + d + 1]
                            f_t = gp.tile([128, NF // 2], DT_H, tag="f")
                            b_t = gp.tile([128, NF // 2], DT_H, tag="b")
                            r_t = gp.tile([128, NF // 2], DT_H, tag="r")
                            w_t = gp.tile([128, NF // 2], DT_H, tag="w")
                            f_v = f_t[:].rearrange("p (n l) -> p n l", l=128)
                            b_v = b_t[:].rearrange("p (n l) -> p n l", l=128)
                            for s4 in range(NSPAN // 2):
                                span = half * (NSPAN // 2) + s4
                                zf_ps = pp.tile([128, 2 * SPAN], F32, name="zf", tag="zf")
                                rh_ps = pp.tile([128, 2 * SPAN], F32, name="rh", tag="rh")
                                pst = [zf_ps[:, 0:SPAN], zf_ps[:, SPAN:2 * SPAN],
                                       rh_ps[:, 0:SPAN], rh_ps[:, SPAN:2 * SPAN]]
                                for o in range(4):
                                    for h2 in range(SPAN // 512):
                                        osl = pst[o][:, h2 * 512:(h2 + 1) * 512]  # noqa
                                        base = span * SPAN + h2 * 512
                                        for ct in range(nct):
                                            if li == 0:
                                                rhs = xn2_t[:, base + 2 * ct:base + 2 * ct + 512]
                                                lhsT = w0_v[:, d, ct, o * 128:(o + 1) * 128]
                                            else:
                                                rhs = hin[ct][:, base:base + 512]
                                                lhsT = wi_v[:, li - 1, d, ct, o * 128:(o + 1) * 128]
                                            nc.tensor.matmul(
                                                osl, lhsT, rhs,
                                                start=(ct == 0), stop=(ct == nct - 1),
                                            )
                                # evacuate span, full 128-blocks. d=1 stores each
                                # block reversed (pads land at l' in [0,7)).
                                ssl = slice(s4 * SPAN, (s4 + 1) * SPAN)
                                if d == 0:
                                    srcs = list(pst)
                                else:
                                    srcs = [pq.rearrange("p (n l) -> p n l", l=128)[:, :, ::-1]
                                            for pq in pst]
                                # g = 1-f = sigmoid(-(x+bf)); bfp holds -bf
                                nc.scalar.activation(f_t[:, ssl], srcs[1], sig,
                                                     bias=bcol, scale=-1.0)
                                nc.scalar.activation(r_t[:, ssl], srcs[2], sig, bias=rcol)
                                nc.scalar.copy(b_t[:, ssl], srcs[0])   # z
                                nc.scalar.copy(w_t[:, ssl], srcs[3])   # hp
                            # b'' = g*z (in place over z; must read g before
                            # the 1-g pass below overwrites it)
                            nc.vector.tensor_mul(b_t[:], f_t[:], b_t[:])
                            # f = 1 - g  (tensor_scalar, 4x mode)
                            nc.vector.tensor_scalar(f_t[:], f_t[:], -1.0, 1.0,
                                                    op0=OP.mult, op1=OP.add)
                            # pads reset the scan carry between sequences
                            pads = slice(121, 128) if d == 0 else slice(0, 7)
                            nc.gpsimd.memset(f_v[:, :, pads], 0.0)
                            nc.gpsimd.memset(b_v[:, :, pads], 0.0)
                            # c = f*c + (1-f)*z
                            nc.vector.tensor_tensor_scan(
                                b_t[:], f_t[:], b_t[:], 0.0,
                                op0=OP.mult, op1=OP.add,
                            )
                            # highway: out = r*(cs-hp) + hp
                            nc.vector.tensor_sub(f_t[:], b_t[:], w_t[:])
                            nc.vector.tensor_mul(r_t[:], r_t[:], f_t[:])
                            hov = hout[d][:, ooff:ooff + NF].rearrange(
                                "p (n l) -> p n l", l=128
                            )
                            dst = hov[:, half * 32:half * 32 + 32, :]
                            if d == 1:
                                dst = dst[:, :, ::-1]
                            r_v = r_t[:].rearrange("p (n l) -> p n l", l=128)
                            w_v = w_t[:].rearrange("p (n l) -> p n l", l=128)
                            nc.vector.tensor_add(dst, r_v[:, :, :], w_v[:, :, :])

            # ================= transposed conv + residual =================
            h4 = [h_t[2], h_t[3]]  # layer 3 writes pair B at offset 8
            for t4 in h4:
                v = t4[:, 0:NF].rearrange("p (n l) -> p n l", l=128)
                nc.gpsimd.memset(t4[:, 0:8], 0.0)
                nc.gpsimd.memset(v[:, 1:33, 1:8], 0.0)
                nc.gpsimd.memset(v[:, 33:64, 1:8], 0.0)
                nc.gpsimd.memset(t4[:, NF + 1:XCOLS], 0.0)
            with (
                tc.tile_pool(name="cvp", bufs=4, space="PSUM") as cvp,
                tc.tile_pool(name="osp", bufs=2) as osp,
            ):
                for span in range(NSPAN):
                    c_ps = cvp.tile([C, SPAN], F32, tag="c")
                    for h2 in range(SPAN // 512):
                        osl = c_ps[:, h2 * 512:(h2 + 1) * 512]
                        base = span * SPAN + h2 * 512
                        mm = 0
                        for ct in range(2):
                            for k in range(8):
                                rhs = h4[ct][:, 8 - k + base:8 - k + base + 512]
                                nc.tensor.matmul(
                                    osl, cw_v[:, ct, k, :], rhs,
                                    start=(mm == 0), stop=(mm == 15),
                                )
                                mm += 1
                    o_t = osp.tile([C, SPAN], F32, tag="o")
                    sl = slice(span * SPAN, (span + 1) * SPAN)
                    nc.vector.scalar_tensor_tensor(
                        o_t[:], c_ps[:], cb_t[:, 0:1], xn_t[:, sl],
                        op0=OP.add, op1=OP.add,
                    )
                    nc.sync.dma_start(out_d[:, sl], o_t[:])

    nc.compile()
    return nc


def _prep_weights(W0, Ws, convW):
    w0r = W0.reshape(C, K, 2, 4 * H)
    w0p = np.zeros((2, 4, 128, 512), np.float32)
    for d in range(2):
        for kp in range(4):
            w0p[d, kp, 0:64] = w0r[:, 2 * kp, d]
            w0p[d, kp, 64:128] = w0r[:, 2 * kp + 1, d]
    wip = np.zeros((3, 2, 2, 128, 512), np.float32)
    for i in range(3):
        for d in range(2):
            for ct in range(2):
                wip[i, d, ct] = Ws[i][ct * 128:(ct + 1) * 128, d]
    cwp = np.zeros((2, 8, 128, C), np.float32)
    for ct in range(2):
        for k in range(8):
            cwp[ct, k] = convW[ct * 128:(ct + 1) * 128, :, k]
    bf16 = ml_dtypes.bfloat16
    return w0p.astype(bf16), wip.astype(bf16), cwp.astype(bf16)


def kernel(**inputs):
    inputs = {k: np.asarray(v) for k, v in inputs.items()}
    x = inputs["x"].astype(np.float32)
    xs = np.ascontiguousarray(
        x.transpose(0, 2, 1, 3).reshape(B * T, C, F_)
    )  # (512, C, F)

    w0p, wip, cwp = _prep_weights(
        inputs["W0"].astype(np.float32),
        [inputs[f"W{i}"].astype(np.float32) for i in (1, 2, 3)],
        inputs["convW"].astype(np.float32),
    )
    bfp = -np.stack([inputs[f"bf{i}"] for i in range(4)]).astype(np.float32)
    brp = np.stack([inputs[f"br{i}"] for i in range(4)]).astype(np.float32)
    gm = inputs["gamma"].reshape(C).astype(np.float32)
    bt = inputs["beta"].reshape(C).astype(np.float32)
    cb = inputs["convb"].reshape(C).astype(np.float32)

    if "nc" not in _CACHE:
        _CACHE["nc"] = _build()
    nc = _CACHE["nc"]

    shared = {"w0p": w0p, "wip": wip, "cwp": cwp, "bfp": bfp, "brp": brp,
              "gm": gm, "bt": bt, "cb": cb}
    in_maps = []
    for core in range(NCORES):
        sh = xs[core * NLOC:(core + 1) * NLOC]  # (NLOC, C, F)
        u = np.ascontiguousarray(sh.transpose(1, 0, 2))  # (C, NLOC, F)
        un = np.ascontiguousarray(sh)
        in_maps.append({"u": u, "un": un, **shared})

    trace = bool(os.environ.get("KBENCH_TRACE"))
    res = bass_utils.run_bass_kernel_spmd(
        nc, in_maps, list(range(NCORES)), trace=trace,
        tmpdir=os.environ.get("KBENCH_TMPDIR"),
    )
    _CACHE["last_result"] = res

    full = np.concatenate(
        [res.results[i]["o"].reshape(C, NLOC, F_) for i in range(NCORES)], axis=1
    )  # (C, 512, F)
    out = full.transpose(1, 0, 2).reshape(B, T, C, F_).transpose(0, 2, 1, 3)
    return np.ascontiguousarray(out.astype(np.float32))
